# revision 13
# baseline (speedup 1.0000x reference)
"""Trainium2 Bass kernel for the CNF reversible backward solve.

Math restructuring (exact, validated in fp64 against the jax reference):

The per-step recursion is tracked purely in H-space (H=256) via
Z = W1 z, Y = W1 y:
    a_even = tanh(Y + beta_even)
    Z     += Mz @ a_even                       (Mz = -h W1 W2)
    a_odd  = tanh(Z + beta_odd)
    Y'     = inv_l Y + (1-inv_l) Z + inv_l Mz @ a_odd

On device both states live in PSUM banks and are updated by matmuls only:
  Z-bank carries Z + beta_odd(s) (biases injected by tiny rank-2 matmuls),
  so a_odd = tanh(Z-bank) directly.
  Y-bank carries Yhat_s = l*(Y_s + beta_even_s); with this normalization the
  update has STEP-CONSTANT weights:
      Yhat_{s+1} = Mz @ a_odd + (inv_l*I) @ p + rank2(delta_s)
      p = l(l-1) * Zbank + Yhat_s            (one fused DVE op, off-path)
  and a_even = tanh(inv_l * Y-bank).

The device streams all activations a_e to DRAM; the D-space outputs are
exact fp64 host-side postprocessing:
    y_final = c_y y1 + sum_e gamma_e (W2 @ a_e) + c_b b2
    I_final = h (N sum(c) - sum_s c . a_even_s^2),   c = diag(W1 W2)

Sharding: data-parallel, B=256 -> 32 samples on each of 8 cores; parameters
replicated; gather + assembly on host.
"""

import numpy as np
import ml_dtypes
from contextlib import ExitStack

import concourse.bass as bass
import concourse.tile as tile
from concourse import bacc, mybir
from concourse.bass_utils import run_bass_kernel_spmd

# Problem constants (hardcoded per contract)
NCORES = 8
B, D, H = 256, 64, 256
NSTEP = 64
HSTEP = 1.0 / NSTEP
LCOUP = 0.999
INVL = 1.0 / LCOUP
BS = B // NCORES  # 32 samples per core
NBLK = H // 128  # 2 h-blocks
FREE = NBLK * BS  # 64: free size of H-space tiles, layout (blk, sample)
NEVAL = 2 * NSTEP  # 128
ACOLS = NSTEP * FREE  # 4096 columns in each activation stream buffer
DMA_CHUNKS = 8
CSTEPS = NSTEP // DMA_CHUNKS  # steps per out-DMA chunk
CTIL = LCOUP * (LCOUP - 1.0)  # STT scalar on Z-bank

F32 = mybir.dt.float32
BF16 = mybir.dt.bfloat16
BF16NP = ml_dtypes.bfloat16


def _coefficients():
    """Exact fp64 scalar recursions for the output-extraction weights."""
    gamma = np.zeros(NEVAL)
    la = np.zeros(NEVAL)
    alpha_y = alpha_z = 1.0
    nu_y = nu_z = 0.0
    for s in range(NSTEP):
        la[2 * s] += -HSTEP
        nu_z += -HSTEP
        gamma *= INVL
        alpha_y *= INVL
        nu_y *= INVL
        gamma += (1.0 - INVL) * la
        alpha_y += (1.0 - INVL) * alpha_z
        nu_y += (1.0 - INVL) * nu_z
        gamma[2 * s + 1] += -INVL * HSTEP
        nu_y += -INVL * HSTEP
    return gamma, alpha_y, nu_y


def _host_tables(W1, b1, u1, W2, b2):
    """All precomputed tensors, fp64 internally."""
    W1 = W1.astype(np.float64)
    W2 = W2.astype(np.float64)
    b1 = b1.astype(np.float64)
    u1 = u1.astype(np.float64)
    b2 = b2.astype(np.float64)

    Mz = -HSTEP * (W1 @ W2)  # [H, H]
    W1b2 = W1 @ b2  # [H]
    l = LCOUP

    def be(s):
        return b1 + (1.0 - s * HSTEP) * u1

    def bp(s):  # beta_odd
        return b1 + (1.0 - (s + 1) * HSTEP) * u1 - (s + 1) * HSTEP * W1b2

    # mzt_pack[p, (k*NBLK+j)*128 + m] = Mz[128*j+m, 128*k+p]
    MzT = Mz.T
    mzt_pack = np.zeros((128, NBLK * NBLK * 128))
    for k in range(NBLK):
        for j in range(NBLK):
            mzt_pack[:, (k * NBLK + j) * 128 : (k * NBLK + j + 1) * 128] = MzT[
                128 * k : 128 * k + 128, 128 * j : 128 * j + 128
            ]

    # rank-2 bias tables: lhsT slice [2, 128] at cols 128*s
    dbz = np.zeros((2, NSTEP * 128))
    dby = np.zeros((2, NSTEP * 128))
    for s in range(NSTEP):
        dz = bp(s) if s == 0 else bp(s) - bp(s - 1)
        for k in range(NBLK):
            dbz[k, s * 128 : (s + 1) * 128] = dz[128 * k : 128 * k + 128]
    for s in range(NSTEP - 1):
        dh = -HSTEP * W1b2 + l * be(s + 1) - (l - 1.0) * bp(s) - be(s)
        for k in range(NBLK):
            dby[k, s * 128 : (s + 1) * 128] = dh[128 * k : 128 * k + 128]
    # col-block NSTEP-1 of dby = init bias l*be(0)
    ib = l * be(0)
    for k in range(NBLK):
        dby[k, (NSTEP - 1) * 128 : NSTEP * 128] = ib[128 * k : 128 * k + 128]

    ind = np.zeros((2, FREE))
    for k in range(NBLK):
        ind[k, k * BS : (k + 1) * BS] = 1.0

    jid = INVL * np.eye(128)

    return dict(
        mzt=mzt_pack.astype(BF16NP),
        dbz=dbz.astype(np.float32),
        dby=dby.astype(np.float32),
        ind=ind.astype(np.float32),
        jid=jid.astype(np.float32),
        w1t=W1.T.astype(np.float32),
        w1tl=(l * W1.T).astype(np.float32),
    )


def _build_kernel():
    """Build the Bass module (same program for every core)."""
    nc = bacc.Bacc("TRN2", target_bir_lowering=False, debug=False)

    y1t_d = nc.dram_tensor("y1t", [D, BS], F32, kind="ExternalInput").ap()
    w1t_d = nc.dram_tensor("w1t", [D, H], F32, kind="ExternalInput").ap()
    w1tl_d = nc.dram_tensor("w1tl", [D, H], F32, kind="ExternalInput").ap()
    mzt_d = nc.dram_tensor("mzt", [128, NBLK * NBLK * 128], BF16, kind="ExternalInput").ap()
    dbz_d = nc.dram_tensor("dbz", [2, NSTEP * 128], F32, kind="ExternalInput").ap()
    dby_d = nc.dram_tensor("dby", [2, NSTEP * 128], F32, kind="ExternalInput").ap()
    ind_d = nc.dram_tensor("ind", [2, FREE], F32, kind="ExternalInput").ap()
    jid_d = nc.dram_tensor("jid", [128, 128], F32, kind="ExternalInput").ap()

    ae_out_d = nc.dram_tensor("ae_out", [128, ACOLS], BF16, kind="ExternalOutput").ap()
    ao_out_d = nc.dram_tensor("ao_out", [128, ACOLS], BF16, kind="ExternalOutput").ap()

    with tile.TileContext(nc) as tc, ExitStack() as ctx:
        consts = ctx.enter_context(tc.tile_pool(name="consts", bufs=1))
        zpool = ctx.enter_context(tc.tile_pool(name="zps", bufs=1, space="PSUM"))
        ypool = ctx.enter_context(tc.tile_pool(name="yps", bufs=2, space="PSUM"))
        ppool = ctx.enter_context(tc.tile_pool(name="ptmp", bufs=2))

        # --- prime the tanh activation table early (dep-free) ---
        warm = consts.tile([1, 8], F32, tag="warm")
        nc.vector.memset(warm[:], 0.0)
        nc.scalar.activation(warm[:], warm[:], mybir.ActivationFunctionType.Tanh)

        # --- load constants ---
        y1t = consts.tile([D, BS], F32, tag="y1t")
        nc.sync.dma_start(y1t[:], y1t_d)
        w1t = consts.tile([D, H], F32, tag="w1t")
        nc.sync.dma_start(w1t[:], w1t_d)
        w1tl = consts.tile([D, H], F32, tag="w1tl")
        nc.sync.dma_start(w1tl[:], w1tl_d)
        mzt = consts.tile([128, NBLK * NBLK * 128], BF16, tag="mzt")
        nc.sync.dma_start(mzt[:], mzt_d)
        dbz = consts.tile([2, NSTEP * 128], F32, tag="dbz")
        nc.sync.dma_start(dbz[:], dbz_d)
        dby = consts.tile([2, NSTEP * 128], F32, tag="dby")
        nc.sync.dma_start(dby[:], dby_d)
        ind = consts.tile([2, FREE], F32, tag="ind")
        nc.sync.dma_start(ind[:], ind_d)
        jid = consts.tile([128, 128], F32, tag="jid")
        nc.sync.dma_start(jid[:], jid_d)

        # --- activation stream buffers ---
        abuf_e = consts.tile([128, ACOLS], BF16, tag="abuf_e")
        abuf_o = consts.tile([128, ACOLS], BF16, tag="abuf_o")

        def mzt_blk(k, j):
            base = (k * NBLK + j) * 128
            return mzt[:, base : base + 128]

        z_ps = zpool.tile([128, FREE], F32, tag="z")

        # --- init Z-bank = W1 @ y1 + beta_odd(0) ---
        for j in range(NBLK):
            nc.tensor.matmul(
                z_ps[:, j * BS : (j + 1) * BS],
                w1t[:, 128 * j : 128 * j + 128],
                y1t[:],
                start=(j == 0),
                stop=False,
            )
        nc.tensor.matmul(
            z_ps[:], dbz[:, 0:128], ind[:], start=False, stop=True
        )

        # --- init Y-bank = l*(W1 @ y1) + l*be(0) ---
        y_cur = ypool.tile([128, FREE], F32, tag="y")
        for j in range(NBLK):
            nc.tensor.matmul(
                y_cur[:, j * BS : (j + 1) * BS],
                w1tl[:, 128 * j : 128 * j + 128],
                y1t[:],
                start=(j == 0),
                stop=False,
            )
        nc.tensor.matmul(
            y_cur[:], dby[:, (NSTEP - 1) * 128 : NSTEP * 128], ind[:],
            start=False, stop=True,
        )

        for s in range(NSTEP):
            last = s == NSTEP - 1
            ecol = s * FREE

            # Z-bank bias delta for THIS step's odd read (emitted first: runs
            # during the preceding work; serialized after last step's readers)
            if s > 0:
                nc.tensor.matmul(
                    z_ps[:], dbz[:, s * 128 : (s + 1) * 128], ind[:],
                    start=False, stop=False, skip_group_check=True,
                )

            # --- even eval: a_even = tanh(inv_l * Ybank) ---
            a_even = abuf_e[:, ecol : ecol + FREE]
            for blk in range(NBLK):
                nc.scalar.activation(
                    a_even[:, blk * BS : (blk + 1) * BS],
                    y_cur[:, blk * BS : (blk + 1) * BS],
                    mybir.ActivationFunctionType.Tanh,
                    scale=INVL,
                )

            # --- Z += Mz @ a_even ---
            for j in range(NBLK):
                for k in range(NBLK):
                    nc.tensor.matmul(
                        z_ps[:, j * BS : (j + 1) * BS],
                        mzt_blk(k, j),
                        a_even[:, k * BS : (k + 1) * BS],
                        start=False,
                        stop=False,
                        skip_group_check=True,
                    )

            if not last:
                # open next Y-bank with its bias (hidden, dep-free)
                y_next = ypool.tile([128, FREE], F32, tag="y")
                nc.tensor.matmul(
                    y_next[:], dby[:, s * 128 : (s + 1) * 128], ind[:],
                    start=True, stop=False,
                )

            # --- odd eval: a_odd = tanh(Zbank) ---
            a_odd = abuf_o[:, ecol : ecol + FREE]
            for blk in range(NBLK):
                nc.scalar.activation(
                    a_odd[:, blk * BS : (blk + 1) * BS],
                    z_ps[:, blk * BS : (blk + 1) * BS],
                    mybir.ActivationFunctionType.Tanh,
                    scale=1.0,
                )

            if not last:
                # --- p = ctil * Zbank + Ybank (DVE, during/after odd ACT) ---
                # (two ops: DVE cannot read two PSUM operands in one instr)
                t_t = ppool.tile([128, FREE], F32, tag="t")
                nc.vector.tensor_scalar_mul(t_t[:], z_ps[:], CTIL)
                p_t = ppool.tile([128, FREE], F32, tag="p")
                nc.vector.tensor_tensor(
                    p_t[:], t_t[:], y_cur[:], mybir.AluOpType.add
                )

                # --- Ynext += Mz @ a_odd ---
                for j in range(NBLK):
                    for k in range(NBLK):
                        nc.tensor.matmul(
                            y_next[:, j * BS : (j + 1) * BS],
                            mzt_blk(k, j),
                            a_odd[:, k * BS : (k + 1) * BS],
                            start=False,
                            stop=False,
                        )
                # --- Ynext += (inv_l I) @ p ---
                nc.tensor.matmul(
                    y_next[:], jid[:], p_t[:], start=False, stop=True
                )
                y_cur = y_next

            # --- stream out completed chunks ---
            if (s + 1) % CSTEPS == 0:
                c0 = (s + 1 - CSTEPS) * FREE
                c1 = (s + 1) * FREE
                nc.sync.dma_start(ae_out_d[:, c0:c1], abuf_e[:, c0:c1])
                nc.sync.dma_start(ao_out_d[:, c0:c1], abuf_o[:, c0:c1])

    nc.compile()
    return nc


_CACHE = {}


def _get_kernel():
    if "nc" not in _CACHE:
        _CACHE["nc"] = _build_kernel()
    return _CACHE["nc"]


def kernel(y1, W1, b1, u1, W2, b2, _trace=False, _trace_kwargs=None):
    y1 = np.asarray(y1)
    in_dtype = y1.dtype
    W1_ = np.asarray(W1, dtype=np.float64)
    W2_ = np.asarray(W2, dtype=np.float64)
    b2_ = np.asarray(b2, dtype=np.float64)
    tabs = _host_tables(
        np.asarray(W1), np.asarray(b1), np.asarray(u1), np.asarray(W2), np.asarray(b2)
    )

    nc = _get_kernel()

    shared = {k: tabs[k] for k in ["w1t", "w1tl", "mzt", "dbz", "dby", "ind", "jid"]}
    in_maps = []
    for c in range(NCORES):
        shard = y1[c * BS : (c + 1) * BS].astype(np.float32)  # [BS, D]
        m = dict(shared)
        m["y1t"] = np.ascontiguousarray(shard.T)  # [D, BS]
        in_maps.append(m)

    kw = {}
    if _trace:
        kw["trace"] = True
        if _trace_kwargs:
            kw.update(_trace_kwargs)
    res = run_bass_kernel_spmd(nc, in_maps, core_ids=list(range(NCORES)), **kw)

    # --- exact host-side output extraction ---
    gamma, c_y, c_b = _coefficients()
    cvec = np.sum(W1_ * W2_.T, axis=1)  # diag(W1@W2)
    sum_c = float(np.sum(cvec))

    out = np.zeros((B, D + 1), dtype=np.float32)
    for c in range(NCORES):
        ae = np.asarray(res.results[c]["ae_out"]).astype(np.float64)  # [128, ACOLS]
        ao = np.asarray(res.results[c]["ao_out"]).astype(np.float64)
        ae = ae.reshape(128, NSTEP, NBLK, BS)  # [p, s, blk, b]
        ao = ao.reshape(128, NSTEP, NBLK, BS)
        ae = np.moveaxis(ae, (2, 0), (1, 2)).reshape(NSTEP, H, BS)  # [s, h, b]
        ao = np.moveaxis(ao, (2, 0), (1, 2)).reshape(NSTEP, H, BS)

        S = np.einsum("s,shb->hb", gamma[0::2], ae) + np.einsum(
            "s,shb->hb", gamma[1::2], ao
        )
        shard = y1[c * BS : (c + 1) * BS].astype(np.float64)  # [BS, D]
        y_fin = c_y * shard + (W2_ @ S).T + c_b * b2_[None, :]
        ptr = np.einsum("h,shb->b", cvec, ae**2)
        i_fin = HSTEP * (NSTEP * sum_c - ptr)
        out[c * BS : (c + 1) * BS, :D] = y_fin.astype(np.float32)
        out[c * BS : (c + 1) * BS, D] = i_fin.astype(np.float32)

    if _trace:
        return out.astype(in_dtype, copy=False), res
    return out.astype(in_dtype, copy=False)


# revision 24
# speedup vs baseline: 1.2495x; 1.2495x over previous
"""Trainium2 Bass kernel for the CNF reversible backward solve.

Math restructuring (exact, validated in fp64 against the jax reference):

The per-step recursion is tracked purely in H-space (H=256) via
Z = W1 z, Y = W1 y:
    a_even = tanh(Y + beta_even)
    Z     += Mz @ a_even                       (Mz = -h W1 W2)
    a_odd  = tanh(Z + beta_odd)
    Y'     = inv_l Y + (1-inv_l) Z + inv_l Mz @ a_odd

On device both states live in PSUM banks and are updated by matmuls only:
  Z-bank carries Z + beta_odd(s) (biases injected by tiny rank-2 matmuls),
  so a_odd = tanh(Z-bank) directly.
  Y-bank carries Yhat_s = l*(Y_s + beta_even_s); with this normalization the
  update has STEP-CONSTANT weights:
      Yhat_{s+1} = Mz @ a_odd + (inv_l*I) @ p + rank2(delta_s)
      p = l(l-1) * Zbank + Yhat_s            (one fused DVE op, off-path)
  and a_even = tanh(inv_l * Y-bank).

The device streams all activations a_e to DRAM; the D-space outputs are
exact fp64 host-side postprocessing:
    y_final = c_y y1 + sum_e gamma_e (W2 @ a_e) + c_b b2
    I_final = h (N sum(c) - sum_s c . a_even_s^2),   c = diag(W1 W2)

Sharding: data-parallel, B=256 -> 32 samples on each of 8 cores; parameters
replicated; gather + assembly on host.
"""

import numpy as np
import ml_dtypes
from contextlib import ExitStack

import concourse.bass as bass
import concourse.tile as tile
from concourse import bacc, mybir
from concourse.bass_utils import run_bass_kernel_spmd

# Problem constants (hardcoded per contract)
NCORES = 8
B, D, H = 256, 64, 256
NSTEP = 64
HSTEP = 1.0 / NSTEP
LCOUP = 0.999
INVL = 1.0 / LCOUP
BS = B // NCORES  # 32 samples per core
NBLK = H // 128  # 2 h-blocks
FREE = NBLK * BS  # 64: free size of H-space tiles, layout (blk, sample)
NEVAL = 2 * NSTEP  # 128
ACOLS = NSTEP * FREE  # 4096 columns in each activation stream buffer
DMA_CHUNKS = 8
CSTEPS = NSTEP // DMA_CHUNKS  # steps per out-DMA chunk
CTIL = LCOUP * (LCOUP - 1.0)  # STT scalar on Z-bank

F32 = mybir.dt.float32
BF16 = mybir.dt.bfloat16
BF16NP = ml_dtypes.bfloat16

SHARED_INPUTS = ["w1t", "w1tl", "mzt", "dbz", "dby", "dbz0", "dby0", "ind", "indb"]


def _coefficients():
    """Exact fp64 scalar recursions for the output-extraction weights."""
    gamma = np.zeros(NEVAL)
    la = np.zeros(NEVAL)
    alpha_y = alpha_z = 1.0
    nu_y = nu_z = 0.0
    for s in range(NSTEP):
        la[2 * s] += -HSTEP
        nu_z += -HSTEP
        gamma *= INVL
        alpha_y *= INVL
        nu_y *= INVL
        gamma += (1.0 - INVL) * la
        alpha_y += (1.0 - INVL) * alpha_z
        nu_y += (1.0 - INVL) * nu_z
        gamma[2 * s + 1] += -INVL * HSTEP
        nu_y += -INVL * HSTEP
    return gamma, alpha_y, nu_y


def _host_tables(W1, b1, u1, W2, b2):
    """All precomputed tensors, fp64 internally."""
    W1 = W1.astype(np.float64)
    W2 = W2.astype(np.float64)
    b1 = b1.astype(np.float64)
    u1 = u1.astype(np.float64)
    b2 = b2.astype(np.float64)

    Mz = -HSTEP * (W1 @ W2)  # [H, H]
    W1b2 = W1 @ b2  # [H]
    l = LCOUP

    def be(s):
        return b1 + (1.0 - s * HSTEP) * u1

    def bp(s):  # beta_odd
        return b1 + (1.0 - (s + 1) * HSTEP) * u1 - (s + 1) * HSTEP * W1b2

    # mzt_pack[p, (k*NBLK+j)*128 + m] = Mz[128*j+m, 128*k+p]
    MzT = Mz.T
    mzt_pack = np.zeros((128, NBLK * NBLK * 128))
    for k in range(NBLK):
        for j in range(NBLK):
            mzt_pack[:, (k * NBLK + j) * 128 : (k * NBLK + j + 1) * 128] = MzT[
                128 * k : 128 * k + 128, 128 * j : 128 * j + 128
            ]

    # rank-2 bias tables: lhsT slice [2, 128] at cols 128*s
    dbz = np.zeros((2, NSTEP * 128))
    dby = np.zeros((2, NSTEP * 128))
    for s in range(NSTEP):
        dz = bp(s) if s == 0 else bp(s) - bp(s - 1)
        for k in range(NBLK):
            dbz[k, s * 128 : (s + 1) * 128] = dz[128 * k : 128 * k + 128]
    for s in range(NSTEP - 1):
        dh = -HSTEP * W1b2 + l * be(s + 1) - (l - 1.0) * bp(s) - be(s)
        for k in range(NBLK):
            dby[k, s * 128 : (s + 1) * 128] = dh[128 * k : 128 * k + 128]
    # col-block NSTEP-1 of dby = init bias l*be(0)
    ib = l * be(0)
    for k in range(NBLK):
        dby[k, (NSTEP - 1) * 128 : NSTEP * 128] = ib[128 * k : 128 * k + 128]

    ind = np.zeros((2, FREE))
    for k in range(NBLK):
        ind[k, k * BS : (k + 1) * BS] = 1.0

    # init biases stay fp32 (full magnitude); per-step deltas are tiny -> bf16
    dbz0 = dbz[:, 0:128].astype(np.float32)
    dby0 = dby[:, (NSTEP - 1) * 128 : NSTEP * 128].astype(np.float32)

    return dict(
        mzt=mzt_pack.astype(BF16NP),
        dbz=dbz.astype(BF16NP),
        dby=dby.astype(BF16NP),
        dbz0=dbz0,
        dby0=dby0,
        ind=ind.astype(np.float32),
        indb=ind.astype(BF16NP),
        w1t=W1.T.astype(np.float32),
        w1tl=(l * W1.T).astype(np.float32),
    )


def _build_kernel():
    """Build the Bass module (same program for every core)."""
    nc = bacc.Bacc("TRN2", target_bir_lowering=False, debug=False)

    y1t_d = nc.dram_tensor("y1t", [D, BS], F32, kind="ExternalInput").ap()
    w1t_d = nc.dram_tensor("w1t", [D, H], F32, kind="ExternalInput").ap()
    w1tl_d = nc.dram_tensor("w1tl", [D, H], F32, kind="ExternalInput").ap()
    mzt_d = nc.dram_tensor("mzt", [128, NBLK * NBLK * 128], BF16, kind="ExternalInput").ap()
    dbz_d = nc.dram_tensor("dbz", [2, NSTEP * 128], BF16, kind="ExternalInput").ap()
    dby_d = nc.dram_tensor("dby", [2, NSTEP * 128], BF16, kind="ExternalInput").ap()
    dbz0_d = nc.dram_tensor("dbz0", [2, 128], F32, kind="ExternalInput").ap()
    dby0_d = nc.dram_tensor("dby0", [2, 128], F32, kind="ExternalInput").ap()
    ind_d = nc.dram_tensor("ind", [2, FREE], F32, kind="ExternalInput").ap()
    indb_d = nc.dram_tensor("indb", [2, FREE], BF16, kind="ExternalInput").ap()

    ae_out_d = nc.dram_tensor("ae_out", [128, ACOLS], BF16, kind="ExternalOutput").ap()
    ao_out_d = nc.dram_tensor("ao_out", [128, ACOLS], BF16, kind="ExternalOutput").ap()

    with tile.TileContext(nc) as tc, ExitStack() as ctx:
        consts = ctx.enter_context(tc.tile_pool(name="consts", bufs=1))
        zpool = ctx.enter_context(tc.tile_pool(name="zps", bufs=1, space="PSUM"))
        ypool = ctx.enter_context(tc.tile_pool(name="yps", bufs=2, space="PSUM"))
        ppool = ctx.enter_context(tc.tile_pool(name="ptmp", bufs=2))

        # --- prime the tanh activation table early (dep-free) ---
        warm = consts.tile([1, 8], F32, tag="warm")
        nc.vector.memset(warm[:], 0.0)
        nc.scalar.activation(warm[:], warm[:], mybir.ActivationFunctionType.Tanh)

        # --- load constants ---
        y1t = consts.tile([D, BS], F32, tag="y1t")
        nc.sync.dma_start(y1t[:], y1t_d)
        w1t = consts.tile([D, H], F32, tag="w1t")
        nc.sync.dma_start(w1t[:], w1t_d)
        w1tl = consts.tile([D, H], F32, tag="w1tl")
        nc.sync.dma_start(w1tl[:], w1tl_d)
        mzt = consts.tile([128, NBLK * NBLK * 128], BF16, tag="mzt")
        nc.sync.dma_start(mzt[:], mzt_d)
        dbz = consts.tile([2, NSTEP * 128], BF16, tag="dbz")
        nc.sync.dma_start(dbz[:], dbz_d)
        dby = consts.tile([2, NSTEP * 128], BF16, tag="dby")
        nc.sync.dma_start(dby[:], dby_d)
        dbz0 = consts.tile([2, 128], F32, tag="dbz0")
        nc.sync.dma_start(dbz0[:], dbz0_d)
        dby0 = consts.tile([2, 128], F32, tag="dby0")
        nc.sync.dma_start(dby0[:], dby0_d)
        ind = consts.tile([2, FREE], F32, tag="ind")
        nc.sync.dma_start(ind[:], ind_d)
        indb = consts.tile([2, FREE], BF16, tag="indb")
        nc.sync.dma_start(indb[:], indb_d)

        # --- activation stream buffers ---
        abuf_e = consts.tile([128, ACOLS], BF16, tag="abuf_e")
        abuf_o = consts.tile([128, ACOLS], BF16, tag="abuf_o")

        def mzt_blk(k, j):
            base = (k * NBLK + j) * 128
            return mzt[:, base : base + 128]

        z_ps = zpool.tile([128, FREE], F32, tag="z")

        # --- init Z-bank = W1 @ y1 + beta_odd(0) ---
        for j in range(NBLK):
            nc.tensor.matmul(
                z_ps[:, j * BS : (j + 1) * BS],
                w1t[:, 128 * j : 128 * j + 128],
                y1t[:],
                start=(j == 0),
                stop=False,
            )
        nc.tensor.matmul(z_ps[:], dbz0[:], ind[:], start=False, stop=True)

        # --- init Y-bank = l*(W1 @ y1) + l*be(0) ---
        y_cur = ypool.tile([128, FREE], F32, tag="y")
        for j in range(NBLK):
            nc.tensor.matmul(
                y_cur[:, j * BS : (j + 1) * BS],
                w1tl[:, 128 * j : 128 * j + 128],
                y1t[:],
                start=(j == 0),
                stop=False,
            )
        nc.tensor.matmul(y_cur[:], dby0[:], ind[:], start=False, stop=True)

        for s in range(NSTEP):
            last = s == NSTEP - 1
            ecol = s * FREE

            # Z-bank bias delta for THIS step's odd read (emitted first: runs
            # during the preceding work; serialized after last step's readers)
            if s > 0:
                nc.tensor.matmul(
                    z_ps[:], dbz[:, s * 128 : (s + 1) * 128], indb[:],
                    start=False, stop=False, skip_group_check=True,
                )

            # --- even eval: a_even = tanh(inv_l * Ybank) ---
            a_even = abuf_e[:, ecol : ecol + FREE]
            for blk in range(NBLK):
                nc.scalar.activation(
                    a_even[:, blk * BS : (blk + 1) * BS],
                    y_cur[:, blk * BS : (blk + 1) * BS],
                    mybir.ActivationFunctionType.Tanh,
                    scale=INVL,
                )

            # --- Z += Mz @ a_even ---
            for j in range(NBLK):
                for k in range(NBLK):
                    nc.tensor.matmul(
                        z_ps[:, j * BS : (j + 1) * BS],
                        mzt_blk(k, j),
                        a_even[:, k * BS : (k + 1) * BS],
                        start=False,
                        stop=False,
                        skip_group_check=True,
                    )

            if not last:
                # open next Y-bank with its bias (hidden, dep-free)
                y_next = ypool.tile([128, FREE], F32, tag="y")
                nc.tensor.matmul(
                    y_next[:], dby[:, s * 128 : (s + 1) * 128], indb[:],
                    start=True, stop=False,
                )

            # --- odd eval: a_odd = tanh(Zbank) ---
            a_odd = abuf_o[:, ecol : ecol + FREE]
            for blk in range(NBLK):
                nc.scalar.activation(
                    a_odd[:, blk * BS : (blk + 1) * BS],
                    z_ps[:, blk * BS : (blk + 1) * BS],
                    mybir.ActivationFunctionType.Tanh,
                    scale=1.0,
                )

            if not last:
                # --- p = inv_l * (ctil * Zbank + Ybank) (DVE, during odd ACT;
                # two stages: DVE cannot read two PSUM operands in one instr) ---
                t_t = ppool.tile([128, FREE], F32, tag="t")
                nc.vector.tensor_scalar_mul(t_t[:], z_ps[:], LCOUP - 1.0)
                p_t = ppool.tile([128, FREE], F32, tag="p")
                nc.vector.scalar_tensor_tensor(
                    p_t[:], y_cur[:], INVL, t_t[:],
                    mybir.AluOpType.mult, mybir.AluOpType.add,
                )

                # --- Ynext += Mz @ a_odd ---
                for j in range(NBLK):
                    for k in range(NBLK):
                        nc.tensor.matmul(
                            y_next[:, j * BS : (j + 1) * BS],
                            mzt_blk(k, j),
                            a_odd[:, k * BS : (k + 1) * BS],
                            start=False,
                            stop=True,
                        )
                # --- Ynext += p (in-place DVE add; carries the Y state) ---
                nc.vector.tensor_tensor(
                    y_next[:], y_next[:], p_t[:], mybir.AluOpType.add
                )
                y_cur = y_next

            # --- stream out completed chunks ---
            if (s + 1) % CSTEPS == 0:
                c0 = (s + 1 - CSTEPS) * FREE
                c1 = (s + 1) * FREE
                nc.sync.dma_start(ae_out_d[:, c0:c1], abuf_e[:, c0:c1])
                nc.sync.dma_start(ao_out_d[:, c0:c1], abuf_o[:, c0:c1])

    nc.compile()
    return nc


_CACHE = {}


def _get_kernel():
    if "nc" not in _CACHE:
        _CACHE["nc"] = _build_kernel()
    return _CACHE["nc"]


def kernel(y1, W1, b1, u1, W2, b2, _trace=False, _trace_kwargs=None):
    y1 = np.asarray(y1)
    in_dtype = y1.dtype
    W1_ = np.asarray(W1, dtype=np.float64)
    W2_ = np.asarray(W2, dtype=np.float64)
    b2_ = np.asarray(b2, dtype=np.float64)
    tabs = _host_tables(
        np.asarray(W1), np.asarray(b1), np.asarray(u1), np.asarray(W2), np.asarray(b2)
    )

    nc = _get_kernel()

    shared = {k: tabs[k] for k in SHARED_INPUTS}
    in_maps = []
    for c in range(NCORES):
        shard = y1[c * BS : (c + 1) * BS].astype(np.float32)  # [BS, D]
        m = dict(shared)
        m["y1t"] = np.ascontiguousarray(shard.T)  # [D, BS]
        in_maps.append(m)

    kw = {}
    if _trace:
        kw["trace"] = True
        if _trace_kwargs:
            kw.update(_trace_kwargs)
    res = run_bass_kernel_spmd(nc, in_maps, core_ids=list(range(NCORES)), **kw)

    # --- exact host-side output extraction ---
    gamma, c_y, c_b = _coefficients()
    cvec = np.sum(W1_ * W2_.T, axis=1)  # diag(W1@W2)
    sum_c = float(np.sum(cvec))

    out = np.zeros((B, D + 1), dtype=np.float32)
    for c in range(NCORES):
        ae = np.asarray(res.results[c]["ae_out"]).astype(np.float64)  # [128, ACOLS]
        ao = np.asarray(res.results[c]["ao_out"]).astype(np.float64)
        ae = ae.reshape(128, NSTEP, NBLK, BS)  # [p, s, blk, b]
        ao = ao.reshape(128, NSTEP, NBLK, BS)
        ae = np.moveaxis(ae, (2, 0), (1, 2)).reshape(NSTEP, H, BS)  # [s, h, b]
        ao = np.moveaxis(ao, (2, 0), (1, 2)).reshape(NSTEP, H, BS)

        S = np.einsum("s,shb->hb", gamma[0::2], ae) + np.einsum(
            "s,shb->hb", gamma[1::2], ao
        )
        shard = y1[c * BS : (c + 1) * BS].astype(np.float64)  # [BS, D]
        y_fin = c_y * shard + (W2_ @ S).T + c_b * b2_[None, :]
        ptr = np.einsum("h,shb->b", cvec, ae**2)
        i_fin = HSTEP * (NSTEP * sum_c - ptr)
        out[c * BS : (c + 1) * BS, :D] = y_fin.astype(np.float32)
        out[c * BS : (c + 1) * BS, D] = i_fin.astype(np.float32)

    if _trace:
        return out.astype(in_dtype, copy=False), res
    return out.astype(in_dtype, copy=False)


# revision 25
# speedup vs baseline: 1.2504x; 1.0007x over previous
"""Trainium2 Bass kernel for the CNF reversible backward solve.

Math restructuring (exact, validated in fp64 against the jax reference):

The per-step recursion is tracked purely in H-space (H=256) via
Z = W1 z, Y = W1 y:
    a_even = tanh(Y + beta_even)
    Z     += Mz @ a_even                       (Mz = -h W1 W2)
    a_odd  = tanh(Z + beta_odd)
    Y'     = inv_l Y + (1-inv_l) Z + inv_l Mz @ a_odd

On device both states live in PSUM banks and are updated by matmuls only:
  Z-bank carries Z + beta_odd(s) (biases injected by tiny rank-2 matmuls),
  so a_odd = tanh(Z-bank) directly.
  Y-bank carries Yhat_s = l*(Y_s + beta_even_s); with this normalization the
  update has STEP-CONSTANT weights:
      Yhat_{s+1} = Mz @ a_odd + (inv_l*I) @ p + rank2(delta_s)
      p = l(l-1) * Zbank + Yhat_s            (one fused DVE op, off-path)
  and a_even = tanh(inv_l * Y-bank).

The device streams all activations a_e to DRAM; the D-space outputs are
exact fp64 host-side postprocessing:
    y_final = c_y y1 + sum_e gamma_e (W2 @ a_e) + c_b b2
    I_final = h (N sum(c) - sum_s c . a_even_s^2),   c = diag(W1 W2)

Sharding: data-parallel, B=256 -> 32 samples on each of 8 cores; parameters
replicated; gather + assembly on host.
"""

import numpy as np
import ml_dtypes
from contextlib import ExitStack

import concourse.bass as bass
import concourse.tile as tile
from concourse import bacc, mybir
from concourse.bass_utils import run_bass_kernel_spmd

# Problem constants (hardcoded per contract)
NCORES = 8
B, D, H = 256, 64, 256
NSTEP = 64
HSTEP = 1.0 / NSTEP
LCOUP = 0.999
INVL = 1.0 / LCOUP
BS = B // NCORES  # 32 samples per core
NBLK = H // 128  # 2 h-blocks
FREE = NBLK * BS  # 64: free size of H-space tiles, layout (blk, sample)
NEVAL = 2 * NSTEP  # 128
ACOLS = NSTEP * FREE  # 4096 columns in each activation stream buffer
DMA_CHUNKS = 8
CSTEPS = NSTEP // DMA_CHUNKS  # steps per out-DMA chunk
CTIL = LCOUP * (LCOUP - 1.0)  # STT scalar on Z-bank

F32 = mybir.dt.float32
BF16 = mybir.dt.bfloat16
BF16NP = ml_dtypes.bfloat16

SHARED_INPUTS = ["w1t", "w1tl", "mzt", "dbz", "dby", "dbz0", "dby0", "ind", "indb"]


def _coefficients():
    """Exact fp64 scalar recursions for the output-extraction weights."""
    gamma = np.zeros(NEVAL)
    la = np.zeros(NEVAL)
    alpha_y = alpha_z = 1.0
    nu_y = nu_z = 0.0
    for s in range(NSTEP):
        la[2 * s] += -HSTEP
        nu_z += -HSTEP
        gamma *= INVL
        alpha_y *= INVL
        nu_y *= INVL
        gamma += (1.0 - INVL) * la
        alpha_y += (1.0 - INVL) * alpha_z
        nu_y += (1.0 - INVL) * nu_z
        gamma[2 * s + 1] += -INVL * HSTEP
        nu_y += -INVL * HSTEP
    return gamma, alpha_y, nu_y


def _host_tables(W1, b1, u1, W2, b2):
    """All precomputed tensors, fp64 internally."""
    W1 = W1.astype(np.float64)
    W2 = W2.astype(np.float64)
    b1 = b1.astype(np.float64)
    u1 = u1.astype(np.float64)
    b2 = b2.astype(np.float64)

    Mz = -HSTEP * (W1 @ W2)  # [H, H]
    W1b2 = W1 @ b2  # [H]
    l = LCOUP

    def be(s):
        return b1 + (1.0 - s * HSTEP) * u1

    def bp(s):  # beta_odd
        return b1 + (1.0 - (s + 1) * HSTEP) * u1 - (s + 1) * HSTEP * W1b2

    # mzt_pack[p, (k*NBLK+j)*128 + m] = Mz[128*j+m, 128*k+p]
    MzT = Mz.T
    mzt_pack = np.zeros((128, NBLK * NBLK * 128))
    for k in range(NBLK):
        for j in range(NBLK):
            mzt_pack[:, (k * NBLK + j) * 128 : (k * NBLK + j + 1) * 128] = MzT[
                128 * k : 128 * k + 128, 128 * j : 128 * j + 128
            ]

    # rank-2 bias tables: lhsT slice [2, 128] at cols 128*s
    dbz = np.zeros((2, NSTEP * 128))
    dby = np.zeros((2, NSTEP * 128))
    for s in range(NSTEP):
        dz = bp(s) if s == 0 else bp(s) - bp(s - 1)
        for k in range(NBLK):
            dbz[k, s * 128 : (s + 1) * 128] = dz[128 * k : 128 * k + 128]
    for s in range(NSTEP - 1):
        dh = -HSTEP * W1b2 + l * be(s + 1) - (l - 1.0) * bp(s) - be(s)
        for k in range(NBLK):
            dby[k, s * 128 : (s + 1) * 128] = dh[128 * k : 128 * k + 128]
    # col-block NSTEP-1 of dby = init bias l*be(0)
    ib = l * be(0)
    for k in range(NBLK):
        dby[k, (NSTEP - 1) * 128 : NSTEP * 128] = ib[128 * k : 128 * k + 128]

    ind = np.zeros((2, FREE))
    for k in range(NBLK):
        ind[k, k * BS : (k + 1) * BS] = 1.0

    # init biases stay fp32 (full magnitude); per-step deltas are tiny -> bf16
    dbz0 = dbz[:, 0:128].astype(np.float32)
    dby0 = dby[:, (NSTEP - 1) * 128 : NSTEP * 128].astype(np.float32)

    return dict(
        mzt=mzt_pack.astype(BF16NP),
        dbz=dbz.astype(BF16NP),
        dby=dby.astype(BF16NP),
        dbz0=dbz0,
        dby0=dby0,
        ind=ind.astype(np.float32),
        indb=ind.astype(BF16NP),
        w1t=W1.T.astype(np.float32),
        w1tl=(l * W1.T).astype(np.float32),
    )


def _build_kernel():
    """Build the Bass module (same program for every core)."""
    nc = bacc.Bacc("TRN2", target_bir_lowering=False, debug=False)

    y1t_d = nc.dram_tensor("y1t", [D, BS], F32, kind="ExternalInput").ap()
    w1t_d = nc.dram_tensor("w1t", [D, H], F32, kind="ExternalInput").ap()
    w1tl_d = nc.dram_tensor("w1tl", [D, H], F32, kind="ExternalInput").ap()
    mzt_d = nc.dram_tensor("mzt", [128, NBLK * NBLK * 128], BF16, kind="ExternalInput").ap()
    dbz_d = nc.dram_tensor("dbz", [2, NSTEP * 128], BF16, kind="ExternalInput").ap()
    dby_d = nc.dram_tensor("dby", [2, NSTEP * 128], BF16, kind="ExternalInput").ap()
    dbz0_d = nc.dram_tensor("dbz0", [2, 128], F32, kind="ExternalInput").ap()
    dby0_d = nc.dram_tensor("dby0", [2, 128], F32, kind="ExternalInput").ap()
    ind_d = nc.dram_tensor("ind", [2, FREE], F32, kind="ExternalInput").ap()
    indb_d = nc.dram_tensor("indb", [2, FREE], BF16, kind="ExternalInput").ap()

    ae_out_d = nc.dram_tensor("ae_out", [128, ACOLS], BF16, kind="ExternalOutput").ap()
    ao_out_d = nc.dram_tensor("ao_out", [128, ACOLS], BF16, kind="ExternalOutput").ap()

    with tile.TileContext(nc) as tc, ExitStack() as ctx:
        consts = ctx.enter_context(tc.tile_pool(name="consts", bufs=1))
        zpool = ctx.enter_context(tc.tile_pool(name="zps", bufs=1, space="PSUM"))
        ypool = ctx.enter_context(tc.tile_pool(name="yps", bufs=2, space="PSUM"))
        ppool = ctx.enter_context(tc.tile_pool(name="ptmp", bufs=2))

        # --- prime the tanh activation table early (dep-free) ---
        warm = consts.tile([1, 8], F32, tag="warm")
        nc.vector.memset(warm[:], 0.0)
        nc.scalar.activation(warm[:], warm[:], mybir.ActivationFunctionType.Tanh)

        # --- load constants ---
        y1t = consts.tile([D, BS], F32, tag="y1t")
        nc.sync.dma_start(y1t[:], y1t_d)
        w1t = consts.tile([D, H], F32, tag="w1t")
        nc.sync.dma_start(w1t[:], w1t_d)
        w1tl = consts.tile([D, H], F32, tag="w1tl")
        nc.sync.dma_start(w1tl[:], w1tl_d)
        mzt = consts.tile([128, NBLK * NBLK * 128], BF16, tag="mzt")
        nc.sync.dma_start(mzt[:], mzt_d)
        dbz = consts.tile([2, NSTEP * 128], BF16, tag="dbz")
        nc.sync.dma_start(dbz[:], dbz_d)
        dby = consts.tile([2, NSTEP * 128], BF16, tag="dby")
        nc.sync.dma_start(dby[:], dby_d)
        dbz0 = consts.tile([2, 128], F32, tag="dbz0")
        nc.sync.dma_start(dbz0[:], dbz0_d)
        dby0 = consts.tile([2, 128], F32, tag="dby0")
        nc.sync.dma_start(dby0[:], dby0_d)
        ind = consts.tile([2, FREE], F32, tag="ind")
        nc.sync.dma_start(ind[:], ind_d)
        indb = consts.tile([2, FREE], BF16, tag="indb")
        nc.sync.dma_start(indb[:], indb_d)

        # --- activation stream buffers ---
        abuf_e = consts.tile([128, ACOLS], BF16, tag="abuf_e")
        abuf_o = consts.tile([128, ACOLS], BF16, tag="abuf_o")

        def mzt_blk(k, j):
            base = (k * NBLK + j) * 128
            return mzt[:, base : base + 128]

        z_ps = zpool.tile([128, FREE], F32, tag="z")

        # --- init Z-bank = W1 @ y1 + beta_odd(0) ---
        for j in range(NBLK):
            nc.tensor.matmul(
                z_ps[:, j * BS : (j + 1) * BS],
                w1t[:, 128 * j : 128 * j + 128],
                y1t[:],
                start=(j == 0),
                stop=False,
            )
        nc.tensor.matmul(z_ps[:], dbz0[:], ind[:], start=False, stop=True)

        # --- init Y-bank = l*(W1 @ y1) + l*be(0) ---
        y_cur = ypool.tile([128, FREE], F32, tag="y")
        for j in range(NBLK):
            nc.tensor.matmul(
                y_cur[:, j * BS : (j + 1) * BS],
                w1tl[:, 128 * j : 128 * j + 128],
                y1t[:],
                start=(j == 0),
                stop=False,
            )
        nc.tensor.matmul(y_cur[:], dby0[:], ind[:], start=False, stop=True)

        for s in range(NSTEP):
            last = s == NSTEP - 1
            ecol = s * FREE

            # Z-bank bias delta for THIS step's odd read (emitted first: runs
            # during the preceding work; serialized after last step's readers)
            if s > 0:
                nc.tensor.matmul(
                    z_ps[:], dbz[:, s * 128 : (s + 1) * 128], indb[:],
                    start=False, stop=False, skip_group_check=True,
                )

            # --- even eval: a_even = tanh(inv_l * Ybank) ---
            a_even = abuf_e[:, ecol : ecol + FREE]
            for blk in range(NBLK):
                nc.scalar.activation(
                    a_even[:, blk * BS : (blk + 1) * BS],
                    y_cur[:, blk * BS : (blk + 1) * BS],
                    mybir.ActivationFunctionType.Tanh,
                    scale=INVL,
                )

            # --- Z += Mz @ a_even ---
            for j in range(NBLK):
                for k in range(NBLK):
                    nc.tensor.matmul(
                        z_ps[:, j * BS : (j + 1) * BS],
                        mzt_blk(k, j),
                        a_even[:, k * BS : (k + 1) * BS],
                        start=False,
                        stop=False,
                        skip_group_check=True,
                    )

            if not last:
                # open next Y-bank with its bias (hidden, dep-free)
                y_next = ypool.tile([128, FREE], F32, tag="y")
                nc.tensor.matmul(
                    y_next[:], dby[:, s * 128 : (s + 1) * 128], indb[:],
                    start=True, stop=False,
                )

            # --- odd eval: a_odd = tanh(Zbank) ---
            a_odd = abuf_o[:, ecol : ecol + FREE]
            for blk in range(NBLK):
                nc.scalar.activation(
                    a_odd[:, blk * BS : (blk + 1) * BS],
                    z_ps[:, blk * BS : (blk + 1) * BS],
                    mybir.ActivationFunctionType.Tanh,
                    scale=1.0,
                )

            if not last:
                # --- p = inv_l * (ctil * Zbank + Ybank) (DVE, during odd ACT;
                # two stages: DVE cannot read two PSUM operands in one instr) ---
                t_t = ppool.tile([128, FREE], F32, tag="t")
                nc.vector.tensor_scalar_mul(t_t[:], z_ps[:], LCOUP - 1.0)
                p_t = ppool.tile([128, FREE], F32, tag="p")
                nc.vector.scalar_tensor_tensor(
                    p_t[:], y_cur[:], INVL, t_t[:],
                    mybir.AluOpType.mult, mybir.AluOpType.add,
                )

                # --- Ynext += Mz @ a_odd ---
                for j in range(NBLK):
                    for k in range(NBLK):
                        nc.tensor.matmul(
                            y_next[:, j * BS : (j + 1) * BS],
                            mzt_blk(k, j),
                            a_odd[:, k * BS : (k + 1) * BS],
                            start=False,
                            stop=(j == NBLK - 1 and k == NBLK - 1),
                        )
                # --- Ynext += p (in-place DVE add; carries the Y state) ---
                nc.vector.tensor_tensor(
                    y_next[:], y_next[:], p_t[:], mybir.AluOpType.add
                )
                y_cur = y_next

            # --- stream out completed chunks ---
            if (s + 1) % CSTEPS == 0:
                c0 = (s + 1 - CSTEPS) * FREE
                c1 = (s + 1) * FREE
                nc.sync.dma_start(ae_out_d[:, c0:c1], abuf_e[:, c0:c1])
                nc.sync.dma_start(ao_out_d[:, c0:c1], abuf_o[:, c0:c1])

    nc.compile()
    return nc


_CACHE = {}


def _get_kernel():
    if "nc" not in _CACHE:
        _CACHE["nc"] = _build_kernel()
    return _CACHE["nc"]


def kernel(y1, W1, b1, u1, W2, b2, _trace=False, _trace_kwargs=None):
    y1 = np.asarray(y1)
    in_dtype = y1.dtype
    W1_ = np.asarray(W1, dtype=np.float64)
    W2_ = np.asarray(W2, dtype=np.float64)
    b2_ = np.asarray(b2, dtype=np.float64)
    tabs = _host_tables(
        np.asarray(W1), np.asarray(b1), np.asarray(u1), np.asarray(W2), np.asarray(b2)
    )

    nc = _get_kernel()

    shared = {k: tabs[k] for k in SHARED_INPUTS}
    in_maps = []
    for c in range(NCORES):
        shard = y1[c * BS : (c + 1) * BS].astype(np.float32)  # [BS, D]
        m = dict(shared)
        m["y1t"] = np.ascontiguousarray(shard.T)  # [D, BS]
        in_maps.append(m)

    kw = {}
    if _trace:
        kw["trace"] = True
        if _trace_kwargs:
            kw.update(_trace_kwargs)
    res = run_bass_kernel_spmd(nc, in_maps, core_ids=list(range(NCORES)), **kw)

    # --- exact host-side output extraction ---
    gamma, c_y, c_b = _coefficients()
    cvec = np.sum(W1_ * W2_.T, axis=1)  # diag(W1@W2)
    sum_c = float(np.sum(cvec))

    out = np.zeros((B, D + 1), dtype=np.float32)
    for c in range(NCORES):
        ae = np.asarray(res.results[c]["ae_out"]).astype(np.float64)  # [128, ACOLS]
        ao = np.asarray(res.results[c]["ao_out"]).astype(np.float64)
        ae = ae.reshape(128, NSTEP, NBLK, BS)  # [p, s, blk, b]
        ao = ao.reshape(128, NSTEP, NBLK, BS)
        ae = np.moveaxis(ae, (2, 0), (1, 2)).reshape(NSTEP, H, BS)  # [s, h, b]
        ao = np.moveaxis(ao, (2, 0), (1, 2)).reshape(NSTEP, H, BS)

        S = np.einsum("s,shb->hb", gamma[0::2], ae) + np.einsum(
            "s,shb->hb", gamma[1::2], ao
        )
        shard = y1[c * BS : (c + 1) * BS].astype(np.float64)  # [BS, D]
        y_fin = c_y * shard + (W2_ @ S).T + c_b * b2_[None, :]
        ptr = np.einsum("h,shb->b", cvec, ae**2)
        i_fin = HSTEP * (NSTEP * sum_c - ptr)
        out[c * BS : (c + 1) * BS, :D] = y_fin.astype(np.float32)
        out[c * BS : (c + 1) * BS, D] = i_fin.astype(np.float32)

    if _trace:
        return out.astype(in_dtype, copy=False), res
    return out.astype(in_dtype, copy=False)


# revision 28
# speedup vs baseline: 1.4414x; 1.1527x over previous
"""Trainium2 Bass kernel for the CNF reversible backward solve.

Math restructuring (exact, validated in fp64 against the jax reference):

The per-step recursion is tracked purely in H-space (H=256) via
Z = W1 z, Y = W1 y:
    a_even = tanh(Y + beta_even)
    Z     += Mz @ a_even                       (Mz = -h W1 W2)
    a_odd  = tanh(Z + beta_odd)
    Y'     = inv_l Y + (1-inv_l) Z + inv_l Mz @ a_odd

On device both states live in PSUM banks and are updated by matmuls only:
  Z-bank carries Z + beta_odd(s) (biases injected by tiny rank-2 matmuls),
  so a_odd = tanh(Z-bank) directly.
  Y-bank carries Yhat_s = l*(Y_s + beta_even_s); with this normalization the
  update has STEP-CONSTANT weights:
      Yhat_{s+1} = Mz @ a_odd + (inv_l*I) @ p + rank2(delta_s)
      p = l(l-1) * Zbank + Yhat_s            (one fused DVE op, off-path)
  and a_even = tanh(inv_l * Y-bank).

The device streams all activations a_e to DRAM; the D-space outputs are
exact fp64 host-side postprocessing:
    y_final = c_y y1 + sum_e gamma_e (W2 @ a_e) + c_b b2
    I_final = h (N sum(c) - sum_s c . a_even_s^2),   c = diag(W1 W2)

Sharding: data-parallel, B=256 -> 32 samples on each of 8 cores; parameters
replicated; gather + assembly on host.
"""

import numpy as np
import ml_dtypes
from contextlib import ExitStack

import concourse.bass as bass
import concourse.tile as tile
from concourse import bacc, mybir
from concourse.bass_utils import run_bass_kernel_spmd

# Problem constants (hardcoded per contract)
NCORES = 8
B, D, H = 256, 64, 256
NSTEP = 64
HSTEP = 1.0 / NSTEP
LCOUP = 0.999
INVL = 1.0 / LCOUP
BS = B // NCORES  # 32 samples per core
NBLK = H // 128  # 2 h-blocks
FREE = NBLK * BS  # 64: free size of H-space tiles, layout (blk, sample)
NEVAL = 2 * NSTEP  # 128
ACOLS = NSTEP * FREE  # 4096 columns in each activation stream buffer
DMA_CHUNKS = 8
CSTEPS = NSTEP // DMA_CHUNKS  # steps per out-DMA chunk
CTIL = LCOUP * (LCOUP - 1.0)  # STT scalar on Z-bank

F32 = mybir.dt.float32
BF16 = mybir.dt.bfloat16
BF16NP = ml_dtypes.bfloat16

SHARED_INPUTS = ["w1t", "w1tl", "mzt", "dbz", "dby", "dbz0", "dby0", "ind", "indb"]


def _coefficients():
    """Exact fp64 scalar recursions for the output-extraction weights."""
    gamma = np.zeros(NEVAL)
    la = np.zeros(NEVAL)
    alpha_y = alpha_z = 1.0
    nu_y = nu_z = 0.0
    for s in range(NSTEP):
        la[2 * s] += -HSTEP
        nu_z += -HSTEP
        gamma *= INVL
        alpha_y *= INVL
        nu_y *= INVL
        gamma += (1.0 - INVL) * la
        alpha_y += (1.0 - INVL) * alpha_z
        nu_y += (1.0 - INVL) * nu_z
        gamma[2 * s + 1] += -INVL * HSTEP
        nu_y += -INVL * HSTEP
    return gamma, alpha_y, nu_y


def _host_tables(W1, b1, u1, W2, b2):
    """All precomputed tensors, fp64 internally."""
    W1 = W1.astype(np.float64)
    W2 = W2.astype(np.float64)
    b1 = b1.astype(np.float64)
    u1 = u1.astype(np.float64)
    b2 = b2.astype(np.float64)

    Mz = -HSTEP * (W1 @ W2)  # [H, H]
    W1b2 = W1 @ b2  # [H]
    l = LCOUP

    def be(s):
        return b1 + (1.0 - s * HSTEP) * u1

    def bp(s):  # beta_odd
        return b1 + (1.0 - (s + 1) * HSTEP) * u1 - (s + 1) * HSTEP * W1b2

    # mzt_pack[p, (k*NBLK+j)*128 + m] = Mz[128*j+m, 128*k+p]
    MzT = Mz.T
    mzt_pack = np.zeros((128, NBLK * NBLK * 128))
    for k in range(NBLK):
        for j in range(NBLK):
            mzt_pack[:, (k * NBLK + j) * 128 : (k * NBLK + j + 1) * 128] = MzT[
                128 * k : 128 * k + 128, 128 * j : 128 * j + 128
            ]

    # rank-2 bias tables: lhsT slice [2, 128] at cols 128*s
    dbz = np.zeros((2, NSTEP * 128))
    dby = np.zeros((2, NSTEP * 128))
    for s in range(NSTEP):
        dz = bp(s) if s == 0 else bp(s) - bp(s - 1)
        for k in range(NBLK):
            dbz[k, s * 128 : (s + 1) * 128] = dz[128 * k : 128 * k + 128]
    for s in range(NSTEP - 1):
        dh = -HSTEP * W1b2 + l * be(s + 1) - (l - 1.0) * bp(s) - be(s)
        for k in range(NBLK):
            dby[k, s * 128 : (s + 1) * 128] = dh[128 * k : 128 * k + 128]
    # col-block NSTEP-1 of dby = init bias l*be(0)
    ib = l * be(0)
    for k in range(NBLK):
        dby[k, (NSTEP - 1) * 128 : NSTEP * 128] = ib[128 * k : 128 * k + 128]

    ind = np.zeros((2, FREE))
    for k in range(NBLK):
        ind[k, k * BS : (k + 1) * BS] = 1.0

    # init biases stay fp32 (full magnitude); per-step deltas are tiny -> bf16
    dbz0 = dbz[:, 0:128].astype(np.float32)
    dby0 = dby[:, (NSTEP - 1) * 128 : NSTEP * 128].astype(np.float32)

    return dict(
        mzt=mzt_pack.astype(BF16NP),
        dbz=dbz.astype(BF16NP),
        dby=dby.astype(BF16NP),
        dbz0=dbz0,
        dby0=dby0,
        ind=ind.astype(np.float32),
        indb=ind.astype(BF16NP),
        w1t=W1.T.astype(np.float32),
        w1tl=(l * W1.T).astype(np.float32),
    )


def _build_kernel():
    """Build the Bass module (same program for every core)."""
    nc = bacc.Bacc("TRN2", target_bir_lowering=False, debug=False)

    y1t_d = nc.dram_tensor("y1t", [D, BS], F32, kind="ExternalInput").ap()
    w1t_d = nc.dram_tensor("w1t", [D, H], F32, kind="ExternalInput").ap()
    w1tl_d = nc.dram_tensor("w1tl", [D, H], F32, kind="ExternalInput").ap()
    mzt_d = nc.dram_tensor("mzt", [128, NBLK * NBLK * 128], BF16, kind="ExternalInput").ap()
    dbz_d = nc.dram_tensor("dbz", [2, NSTEP * 128], BF16, kind="ExternalInput").ap()
    dby_d = nc.dram_tensor("dby", [2, NSTEP * 128], BF16, kind="ExternalInput").ap()
    dbz0_d = nc.dram_tensor("dbz0", [2, 128], F32, kind="ExternalInput").ap()
    dby0_d = nc.dram_tensor("dby0", [2, 128], F32, kind="ExternalInput").ap()
    ind_d = nc.dram_tensor("ind", [2, FREE], F32, kind="ExternalInput").ap()
    indb_d = nc.dram_tensor("indb", [2, FREE], BF16, kind="ExternalInput").ap()

    ae_out_d = nc.dram_tensor("ae_out", [128, ACOLS], BF16, kind="ExternalOutput").ap()
    ao_out_d = nc.dram_tensor("ao_out", [128, ACOLS], BF16, kind="ExternalOutput").ap()

    with tile.TileContext(nc) as tc, ExitStack() as ctx:
        consts = ctx.enter_context(tc.tile_pool(name="consts", bufs=1))
        zpool = ctx.enter_context(tc.tile_pool(name="zps", bufs=1, space="PSUM"))
        ypool = ctx.enter_context(tc.tile_pool(name="yps", bufs=2, space="PSUM"))
        ppool = ctx.enter_context(tc.tile_pool(name="ptmp", bufs=2))

        # --- prime the tanh activation table early (dep-free) ---
        warm = consts.tile([1, 8], F32, tag="warm")
        nc.vector.memset(warm[:], 0.0)
        nc.scalar.activation(warm[:], warm[:], mybir.ActivationFunctionType.Tanh)

        # --- load constants ---
        y1t = consts.tile([D, BS], F32, tag="y1t")
        nc.sync.dma_start(y1t[:], y1t_d)
        w1t = consts.tile([D, H], F32, tag="w1t")
        nc.sync.dma_start(w1t[:], w1t_d)
        w1tl = consts.tile([D, H], F32, tag="w1tl")
        nc.sync.dma_start(w1tl[:], w1tl_d)
        mzt = consts.tile([128, NBLK * NBLK * 128], BF16, tag="mzt")
        nc.sync.dma_start(mzt[:], mzt_d)
        dbz = consts.tile([2, NSTEP * 128], BF16, tag="dbz")
        nc.sync.dma_start(dbz[:], dbz_d)
        dby = consts.tile([2, NSTEP * 128], BF16, tag="dby")
        nc.sync.dma_start(dby[:], dby_d)
        dbz0 = consts.tile([2, 128], F32, tag="dbz0")
        nc.sync.dma_start(dbz0[:], dbz0_d)
        dby0 = consts.tile([2, 128], F32, tag="dby0")
        nc.sync.dma_start(dby0[:], dby0_d)
        ind = consts.tile([2, FREE], F32, tag="ind")
        nc.sync.dma_start(ind[:], ind_d)
        indb = consts.tile([2, FREE], BF16, tag="indb")
        nc.sync.dma_start(indb[:], indb_d)

        # --- activation stream buffers ---
        abuf_e = consts.tile([128, ACOLS], BF16, tag="abuf_e")
        abuf_o = consts.tile([128, ACOLS], BF16, tag="abuf_o")

        def mzt_blk(k, j):
            base = (k * NBLK + j) * 128
            return mzt[:, base : base + 128]

        z_ps = zpool.tile([128, FREE], F32, tag="z")

        # --- init Z-bank = W1 @ y1 + beta_odd(0) ---
        for j in range(NBLK):
            nc.tensor.matmul(
                z_ps[:, j * BS : (j + 1) * BS],
                w1t[:, 128 * j : 128 * j + 128],
                y1t[:],
                start=(j == 0),
                stop=False,
            )
        nc.tensor.matmul(z_ps[:], dbz0[:], ind[:], start=False, stop=True)

        # --- init Y-bank = l*(W1 @ y1) + l*be(0) ---
        y_cur = ypool.tile([128, FREE], F32, tag="y")
        for j in range(NBLK):
            nc.tensor.matmul(
                y_cur[:, j * BS : (j + 1) * BS],
                w1tl[:, 128 * j : 128 * j + 128],
                y1t[:],
                start=(j == 0),
                stop=False,
            )
        nc.tensor.matmul(y_cur[:], dby0[:], ind[:], start=False, stop=True)

        for s in range(NSTEP):
            last = s == NSTEP - 1
            ecol = s * FREE

            # Z-bank bias delta for THIS step's odd read (emitted first: runs
            # during the preceding work; serialized after last step's readers)
            if s > 0:
                nc.tensor.matmul(
                    z_ps[:], dbz[:, s * 128 : (s + 1) * 128], indb[:],
                    start=False, stop=False, skip_group_check=True,
                )

            # --- even eval: a_even = tanh(inv_l * Ybank) (single wide ACT) ---
            a_even = abuf_e[:, ecol : ecol + FREE]
            nc.scalar.activation(
                a_even[:], y_cur[:], mybir.ActivationFunctionType.Tanh, scale=INVL
            )

            # --- Z += Mz @ a_even ---
            for j in range(NBLK):
                for k in range(NBLK):
                    nc.tensor.matmul(
                        z_ps[:, j * BS : (j + 1) * BS],
                        mzt_blk(k, j),
                        a_even[:, k * BS : (k + 1) * BS],
                        start=False,
                        stop=False,
                        skip_group_check=True,
                    )

            if not last:
                # open next Y-bank with its bias (hidden, dep-free)
                y_next = ypool.tile([128, FREE], F32, tag="y")
                nc.tensor.matmul(
                    y_next[:], dby[:, s * 128 : (s + 1) * 128], indb[:],
                    start=True, stop=False,
                )

            if not last:
                # --- p = (l-1) Zbank + inv_l Ybank (DVE; emitted before the
                # odd ACT so it runs concurrently — same-bank reads are safe;
                # two stages since DVE reads at most one PSUM operand) ---
                t_t = ppool.tile([128, FREE], F32, tag="t")
                nc.vector.tensor_scalar_mul(t_t[:], z_ps[:], LCOUP - 1.0)
                p_t = ppool.tile([128, FREE], F32, tag="p")
                nc.vector.scalar_tensor_tensor(
                    p_t[:], y_cur[:], INVL, t_t[:],
                    mybir.AluOpType.mult, mybir.AluOpType.add,
                )

            # --- odd eval: a_odd = tanh(Zbank) (single wide ACT) ---
            a_odd = abuf_o[:, ecol : ecol + FREE]
            nc.scalar.activation(
                a_odd[:], z_ps[:], mybir.ActivationFunctionType.Tanh, scale=1.0
            )

            if not last:
                # --- Ynext += Mz @ a_odd ---
                for j in range(NBLK):
                    for k in range(NBLK):
                        nc.tensor.matmul(
                            y_next[:, j * BS : (j + 1) * BS],
                            mzt_blk(k, j),
                            a_odd[:, k * BS : (k + 1) * BS],
                            start=False,
                            stop=(j == NBLK - 1 and k == NBLK - 1),
                        )
                # --- Ynext += p (in-place DVE add; carries the Y state) ---
                nc.vector.tensor_tensor(
                    y_next[:], y_next[:], p_t[:], mybir.AluOpType.add
                )
                y_cur = y_next

            # --- stream out completed chunks ---
            if (s + 1) % CSTEPS == 0:
                c0 = (s + 1 - CSTEPS) * FREE
                c1 = (s + 1) * FREE
                nc.sync.dma_start(ae_out_d[:, c0:c1], abuf_e[:, c0:c1])
                nc.sync.dma_start(ao_out_d[:, c0:c1], abuf_o[:, c0:c1])

    nc.compile()
    return nc


_CACHE = {}


def _get_kernel():
    if "nc" not in _CACHE:
        _CACHE["nc"] = _build_kernel()
    return _CACHE["nc"]


def kernel(y1, W1, b1, u1, W2, b2, _trace=False, _trace_kwargs=None):
    y1 = np.asarray(y1)
    in_dtype = y1.dtype
    W1_ = np.asarray(W1, dtype=np.float64)
    W2_ = np.asarray(W2, dtype=np.float64)
    b2_ = np.asarray(b2, dtype=np.float64)
    tabs = _host_tables(
        np.asarray(W1), np.asarray(b1), np.asarray(u1), np.asarray(W2), np.asarray(b2)
    )

    nc = _get_kernel()

    shared = {k: tabs[k] for k in SHARED_INPUTS}
    in_maps = []
    for c in range(NCORES):
        shard = y1[c * BS : (c + 1) * BS].astype(np.float32)  # [BS, D]
        m = dict(shared)
        m["y1t"] = np.ascontiguousarray(shard.T)  # [D, BS]
        in_maps.append(m)

    kw = {}
    if _trace:
        kw["trace"] = True
        if _trace_kwargs:
            kw.update(_trace_kwargs)
    res = run_bass_kernel_spmd(nc, in_maps, core_ids=list(range(NCORES)), **kw)

    # --- exact host-side output extraction ---
    gamma, c_y, c_b = _coefficients()
    cvec = np.sum(W1_ * W2_.T, axis=1)  # diag(W1@W2)
    sum_c = float(np.sum(cvec))

    out = np.zeros((B, D + 1), dtype=np.float32)
    for c in range(NCORES):
        ae = np.asarray(res.results[c]["ae_out"]).astype(np.float64)  # [128, ACOLS]
        ao = np.asarray(res.results[c]["ao_out"]).astype(np.float64)
        ae = ae.reshape(128, NSTEP, NBLK, BS)  # [p, s, blk, b]
        ao = ao.reshape(128, NSTEP, NBLK, BS)
        ae = np.moveaxis(ae, (2, 0), (1, 2)).reshape(NSTEP, H, BS)  # [s, h, b]
        ao = np.moveaxis(ao, (2, 0), (1, 2)).reshape(NSTEP, H, BS)

        S = np.einsum("s,shb->hb", gamma[0::2], ae) + np.einsum(
            "s,shb->hb", gamma[1::2], ao
        )
        shard = y1[c * BS : (c + 1) * BS].astype(np.float64)  # [BS, D]
        y_fin = c_y * shard + (W2_ @ S).T + c_b * b2_[None, :]
        ptr = np.einsum("h,shb->b", cvec, ae**2)
        i_fin = HSTEP * (NSTEP * sum_c - ptr)
        out[c * BS : (c + 1) * BS, :D] = y_fin.astype(np.float32)
        out[c * BS : (c + 1) * BS, D] = i_fin.astype(np.float32)

    if _trace:
        return out.astype(in_dtype, copy=False), res
    return out.astype(in_dtype, copy=False)


# revision 34
# speedup vs baseline: 1.5430x; 1.0705x over previous
"""Trainium2 Bass kernel for the CNF reversible backward solve.

Math restructuring (exact, validated in fp64 against the jax reference):

The per-step recursion is tracked purely in H-space (H=256) via
Z = W1 z, Y = W1 y:
    a_even = tanh(Y + beta_even)
    Z     += Mz @ a_even                       (Mz = -h W1 W2)
    a_odd  = tanh(Z + beta_odd)
    Y'     = inv_l Y + (1-inv_l) Z + inv_l Mz @ a_odd

On device both states live in PSUM banks and are updated by matmuls only:
  Z-bank carries Z + beta_odd(s) (biases injected by tiny rank-2 matmuls),
  so a_odd = tanh(Z-bank) directly.
  Y-bank carries Yhat_s = l*(Y_s + beta_even_s); with this normalization the
  update has STEP-CONSTANT weights:
      Yhat_{s+1} = Mz @ a_odd + (inv_l*I) @ p + rank2(delta_s)
      p = l(l-1) * Zbank + Yhat_s            (one fused DVE op, off-path)
  and a_even = tanh(inv_l * Y-bank).

The device streams all activations a_e to DRAM; the D-space outputs are
exact fp64 host-side postprocessing:
    y_final = c_y y1 + sum_e gamma_e (W2 @ a_e) + c_b b2
    I_final = h (N sum(c) - sum_s c . a_even_s^2),   c = diag(W1 W2)

Sharding: data-parallel, B=256 -> 32 samples on each of 8 cores; parameters
replicated; gather + assembly on host.
"""

import numpy as np
import ml_dtypes
from contextlib import ExitStack

import concourse.bass as bass
import concourse.tile as tile
from concourse import bacc, mybir
from concourse.bass_utils import run_bass_kernel_spmd

# Problem constants (hardcoded per contract)
NCORES = 8
B, D, H = 256, 64, 256
NSTEP = 64
HSTEP = 1.0 / NSTEP
LCOUP = 0.999
INVL = 1.0 / LCOUP
BS = B // NCORES  # 32 samples per core
NBLK = H // 128  # 2 h-blocks
FREE = NBLK * BS  # 64: free size of H-space tiles, layout (blk, sample)
NEVAL = 2 * NSTEP  # 128
ACOLS = NSTEP * FREE  # 4096 columns in each activation stream buffer
DMA_CHUNKS = 8
CSTEPS = NSTEP // DMA_CHUNKS  # steps per out-DMA chunk
CTIL = LCOUP * (LCOUP - 1.0)  # STT scalar on Z-bank

F32 = mybir.dt.float32
BF16 = mybir.dt.bfloat16
BF16NP = ml_dtypes.bfloat16

SHARED_INPUTS = ["w1t", "w1tl", "mzt", "dbz", "dby", "dbz0", "dby0", "ind", "indb"]


def _coefficients():
    """Exact fp64 scalar recursions for the output-extraction weights."""
    gamma = np.zeros(NEVAL)
    la = np.zeros(NEVAL)
    alpha_y = alpha_z = 1.0
    nu_y = nu_z = 0.0
    for s in range(NSTEP):
        la[2 * s] += -HSTEP
        nu_z += -HSTEP
        gamma *= INVL
        alpha_y *= INVL
        nu_y *= INVL
        gamma += (1.0 - INVL) * la
        alpha_y += (1.0 - INVL) * alpha_z
        nu_y += (1.0 - INVL) * nu_z
        gamma[2 * s + 1] += -INVL * HSTEP
        nu_y += -INVL * HSTEP
    return gamma, alpha_y, nu_y


def _host_tables(W1, b1, u1, W2, b2):
    """All precomputed tensors, fp64 internally."""
    W1 = W1.astype(np.float64)
    W2 = W2.astype(np.float64)
    b1 = b1.astype(np.float64)
    u1 = u1.astype(np.float64)
    b2 = b2.astype(np.float64)

    Mz = -HSTEP * (W1 @ W2)  # [H, H]
    W1b2 = W1 @ b2  # [H]
    l = LCOUP

    def be(s):
        return b1 + (1.0 - s * HSTEP) * u1

    def bp(s):  # beta_odd
        return b1 + (1.0 - (s + 1) * HSTEP) * u1 - (s + 1) * HSTEP * W1b2

    # mzt_pack[p, (k*NBLK+j)*128 + m] = Mz[128*j+m, 128*k+p]
    MzT = Mz.T
    mzt_pack = np.zeros((128, NBLK * NBLK * 128))
    for k in range(NBLK):
        for j in range(NBLK):
            mzt_pack[:, (k * NBLK + j) * 128 : (k * NBLK + j + 1) * 128] = MzT[
                128 * k : 128 * k + 128, 128 * j : 128 * j + 128
            ]

    # rank-2 bias tables: lhsT slice [2, 128] at cols 128*s
    dbz = np.zeros((2, NSTEP * 128))
    dby = np.zeros((2, NSTEP * 128))
    for s in range(NSTEP):
        dz = bp(s) if s == 0 else bp(s) - bp(s - 1)
        for k in range(NBLK):
            dbz[k, s * 128 : (s + 1) * 128] = dz[128 * k : 128 * k + 128]
    for s in range(NSTEP - 1):
        dh = -HSTEP * W1b2 + l * be(s + 1) - (l - 1.0) * bp(s) - be(s)
        for k in range(NBLK):
            dby[k, s * 128 : (s + 1) * 128] = dh[128 * k : 128 * k + 128]
    # col-block NSTEP-1 of dby = init bias l*be(0)
    ib = l * be(0)
    for k in range(NBLK):
        dby[k, (NSTEP - 1) * 128 : NSTEP * 128] = ib[128 * k : 128 * k + 128]

    ind = np.zeros((2, FREE))
    for k in range(NBLK):
        ind[k, k * BS : (k + 1) * BS] = 1.0

    # init biases stay fp32 (full magnitude); per-step deltas are tiny -> bf16
    dbz0 = dbz[:, 0:128].astype(np.float32)
    dby0 = dby[:, (NSTEP - 1) * 128 : NSTEP * 128].astype(np.float32)

    return dict(
        mzt=mzt_pack.astype(BF16NP),
        dbz=dbz.astype(BF16NP),
        dby=dby.astype(BF16NP),
        dbz0=dbz0,
        dby0=dby0,
        ind=ind.astype(np.float32),
        indb=ind.astype(BF16NP),
        w1t=W1.T.astype(np.float32),
        w1tl=(l * W1.T).astype(np.float32),
    )


def _build_kernel():
    """Build the Bass module (same program for every core)."""
    nc = bacc.Bacc("TRN2", target_bir_lowering=False, debug=False)

    y1t_d = nc.dram_tensor("y1t", [D, BS], F32, kind="ExternalInput").ap()
    w1t_d = nc.dram_tensor("w1t", [D, H], F32, kind="ExternalInput").ap()
    w1tl_d = nc.dram_tensor("w1tl", [D, H], F32, kind="ExternalInput").ap()
    mzt_d = nc.dram_tensor("mzt", [128, NBLK * NBLK * 128], BF16, kind="ExternalInput").ap()
    dbz_d = nc.dram_tensor("dbz", [2, NSTEP * 128], BF16, kind="ExternalInput").ap()
    dby_d = nc.dram_tensor("dby", [2, NSTEP * 128], BF16, kind="ExternalInput").ap()
    dbz0_d = nc.dram_tensor("dbz0", [2, 128], F32, kind="ExternalInput").ap()
    dby0_d = nc.dram_tensor("dby0", [2, 128], F32, kind="ExternalInput").ap()
    ind_d = nc.dram_tensor("ind", [2, FREE], F32, kind="ExternalInput").ap()
    indb_d = nc.dram_tensor("indb", [2, FREE], BF16, kind="ExternalInput").ap()

    ae_out_d = nc.dram_tensor("ae_out", [128, ACOLS], BF16, kind="ExternalOutput").ap()
    ao_out_d = nc.dram_tensor("ao_out", [128, ACOLS], BF16, kind="ExternalOutput").ap()

    with tile.TileContext(nc) as tc, ExitStack() as ctx:
        consts = ctx.enter_context(tc.tile_pool(name="consts", bufs=1))
        zpool = ctx.enter_context(tc.tile_pool(name="zps", bufs=1, space="PSUM"))
        ypool = ctx.enter_context(tc.tile_pool(name="yps", bufs=2, space="PSUM"))
        ppool = ctx.enter_context(tc.tile_pool(name="ptmp", bufs=2))

        # --- prime the tanh activation table early (dep-free) ---
        warm = consts.tile([1, 8], F32, tag="warm")
        nc.vector.memset(warm[:], 0.0)
        nc.scalar.activation(warm[:], warm[:], mybir.ActivationFunctionType.Tanh)

        # --- load constants ---
        y1t = consts.tile([D, BS], F32, tag="y1t")
        nc.sync.dma_start(y1t[:], y1t_d)
        w1t = consts.tile([D, H], F32, tag="w1t")
        nc.sync.dma_start(w1t[:], w1t_d)
        w1tl = consts.tile([D, H], F32, tag="w1tl")
        nc.sync.dma_start(w1tl[:], w1tl_d)
        mzt = consts.tile([128, NBLK * NBLK * 128], BF16, tag="mzt")
        nc.sync.dma_start(mzt[:], mzt_d)
        dbz = consts.tile([2, NSTEP * 128], BF16, tag="dbz")
        nc.sync.dma_start(dbz[:], dbz_d)
        dby = consts.tile([2, NSTEP * 128], BF16, tag="dby")
        nc.sync.dma_start(dby[:], dby_d)
        dbz0 = consts.tile([2, 128], F32, tag="dbz0")
        nc.sync.dma_start(dbz0[:], dbz0_d)
        dby0 = consts.tile([2, 128], F32, tag="dby0")
        nc.sync.dma_start(dby0[:], dby0_d)
        ind = consts.tile([2, FREE], F32, tag="ind")
        nc.sync.dma_start(ind[:], ind_d)
        indb = consts.tile([2, FREE], BF16, tag="indb")
        nc.sync.dma_start(indb[:], indb_d)

        # --- activation stream buffers (one tile per DMA chunk so the
        # out-DMA of chunk c never WAR-blocks ACT writes of chunk c+1) ---
        CCOLS = CSTEPS * FREE
        abuf_e = [
            consts.tile([128, CCOLS], BF16, tag=f"abe{c}", name=f"abe{c}")
            for c in range(DMA_CHUNKS)
        ]
        abuf_o = [
            consts.tile([128, CCOLS], BF16, tag=f"abo{c}", name=f"abo{c}")
            for c in range(DMA_CHUNKS)
        ]

        def mzt_blk(k, j):
            base = (k * NBLK + j) * 128
            return mzt[:, base : base + 128]

        z_ps = zpool.tile([128, FREE], F32, tag="z")

        # --- init Z-bank = W1 @ y1 + beta_odd(0) ---
        for j in range(NBLK):
            nc.tensor.matmul(
                z_ps[:, j * BS : (j + 1) * BS],
                w1t[:, 128 * j : 128 * j + 128],
                y1t[:],
                start=(j == 0),
                stop=False,
            )
        nc.tensor.matmul(z_ps[:], dbz0[:], ind[:], start=False, stop=True)

        # --- init Y-bank = l*(W1 @ y1) + l*be(0) ---
        y_cur = ypool.tile([128, FREE], F32, tag="y")
        for j in range(NBLK):
            nc.tensor.matmul(
                y_cur[:, j * BS : (j + 1) * BS],
                w1tl[:, 128 * j : 128 * j + 128],
                y1t[:],
                start=(j == 0),
                stop=False,
            )
        nc.tensor.matmul(y_cur[:], dby0[:], ind[:], start=False, stop=True)

        for s in range(NSTEP):
            last = s == NSTEP - 1
            chunk, cstep = divmod(s, CSTEPS)
            ecol = cstep * FREE

            # Z-bank bias delta for THIS step's odd read (emitted first: runs
            # during the preceding work; serialized after last step's readers)
            if s > 0:
                nc.tensor.matmul(
                    z_ps[:], dbz[:, s * 128 : (s + 1) * 128], indb[:],
                    start=False, stop=False, skip_group_check=True,
                )

            # --- even eval: a_even = tanh(inv_l * Ybank) (single wide ACT) ---
            a_even = abuf_e[chunk][:, ecol : ecol + FREE]
            nc.scalar.activation(
                a_even[:], y_cur[:], mybir.ActivationFunctionType.Tanh, scale=INVL
            )

            # --- Z += Mz @ a_even ---
            for j in range(NBLK):
                for k in range(NBLK):
                    nc.tensor.matmul(
                        z_ps[:, j * BS : (j + 1) * BS],
                        mzt_blk(k, j),
                        a_even[:, k * BS : (k + 1) * BS],
                        start=False,
                        stop=False,
                        skip_group_check=True,
                    )

            if not last:
                # open next Y-bank with its bias (hidden, dep-free)
                y_next = ypool.tile([128, FREE], F32, tag="y")
                nc.tensor.matmul(
                    y_next[:], dby[:, s * 128 : (s + 1) * 128], indb[:],
                    start=True, stop=False,
                )

            # --- odd eval: a_odd = tanh(Zbank) (single wide ACT) ---
            a_odd = abuf_o[chunk][:, ecol : ecol + FREE]
            nc.scalar.activation(
                a_odd[:], z_ps[:], mybir.ActivationFunctionType.Tanh, scale=1.0
            )

            if not last:
                # --- p = (l-1) Zbank + inv_l Ybank (DVE, concurrent with odd
                # ACT; two stages since DVE reads at most one PSUM operand) ---
                t_t = ppool.tile([128, FREE], F32, tag="t")
                nc.vector.tensor_scalar_mul(t_t[:], z_ps[:], LCOUP - 1.0)
                p_t = ppool.tile([128, FREE], F32, tag="p")
                nc.vector.scalar_tensor_tensor(
                    p_t[:], y_cur[:], INVL, t_t[:],
                    mybir.AluOpType.mult, mybir.AluOpType.add,
                )

            if not last:
                # --- Ynext += Mz @ a_odd ---
                for j in range(NBLK):
                    for k in range(NBLK):
                        nc.tensor.matmul(
                            y_next[:, j * BS : (j + 1) * BS],
                            mzt_blk(k, j),
                            a_odd[:, k * BS : (k + 1) * BS],
                            start=False,
                            stop=(j == NBLK - 1 and k == NBLK - 1),
                        )
                # --- Ynext += p (in-place DVE add; carries the Y state) ---
                nc.vector.tensor_tensor(
                    y_next[:], y_next[:], p_t[:], mybir.AluOpType.add
                )
                y_cur = y_next

            # --- stream out completed chunks ---
            if (s + 1) % CSTEPS == 0:
                c0 = chunk * CCOLS
                nc.sync.dma_start(ae_out_d[:, c0 : c0 + CCOLS], abuf_e[chunk][:])
                nc.sync.dma_start(ao_out_d[:, c0 : c0 + CCOLS], abuf_o[chunk][:])

    nc.compile()
    return nc


_CACHE = {}


def _get_kernel():
    if "nc" not in _CACHE:
        _CACHE["nc"] = _build_kernel()
    return _CACHE["nc"]


def kernel(y1, W1, b1, u1, W2, b2, _trace=False, _trace_kwargs=None):
    y1 = np.asarray(y1)
    in_dtype = y1.dtype
    W1_ = np.asarray(W1, dtype=np.float64)
    W2_ = np.asarray(W2, dtype=np.float64)
    b2_ = np.asarray(b2, dtype=np.float64)
    tabs = _host_tables(
        np.asarray(W1), np.asarray(b1), np.asarray(u1), np.asarray(W2), np.asarray(b2)
    )

    nc = _get_kernel()

    shared = {k: tabs[k] for k in SHARED_INPUTS}
    in_maps = []
    for c in range(NCORES):
        shard = y1[c * BS : (c + 1) * BS].astype(np.float32)  # [BS, D]
        m = dict(shared)
        m["y1t"] = np.ascontiguousarray(shard.T)  # [D, BS]
        in_maps.append(m)

    kw = {}
    if _trace:
        kw["trace"] = True
        if _trace_kwargs:
            kw.update(_trace_kwargs)
    res = run_bass_kernel_spmd(nc, in_maps, core_ids=list(range(NCORES)), **kw)

    # --- exact host-side output extraction ---
    gamma, c_y, c_b = _coefficients()
    cvec = np.sum(W1_ * W2_.T, axis=1)  # diag(W1@W2)
    sum_c = float(np.sum(cvec))

    out = np.zeros((B, D + 1), dtype=np.float32)
    for c in range(NCORES):
        ae = np.asarray(res.results[c]["ae_out"]).astype(np.float64)  # [128, ACOLS]
        ao = np.asarray(res.results[c]["ao_out"]).astype(np.float64)
        ae = ae.reshape(128, NSTEP, NBLK, BS)  # [p, s, blk, b]
        ao = ao.reshape(128, NSTEP, NBLK, BS)
        ae = np.moveaxis(ae, (2, 0), (1, 2)).reshape(NSTEP, H, BS)  # [s, h, b]
        ao = np.moveaxis(ao, (2, 0), (1, 2)).reshape(NSTEP, H, BS)

        S = np.einsum("s,shb->hb", gamma[0::2], ae) + np.einsum(
            "s,shb->hb", gamma[1::2], ao
        )
        shard = y1[c * BS : (c + 1) * BS].astype(np.float64)  # [BS, D]
        y_fin = c_y * shard + (W2_ @ S).T + c_b * b2_[None, :]
        ptr = np.einsum("h,shb->b", cvec, ae**2)
        i_fin = HSTEP * (NSTEP * sum_c - ptr)
        out[c * BS : (c + 1) * BS, :D] = y_fin.astype(np.float32)
        out[c * BS : (c + 1) * BS, D] = i_fin.astype(np.float32)

    if _trace:
        return out.astype(in_dtype, copy=False), res
    return out.astype(in_dtype, copy=False)


# revision 41
# speedup vs baseline: 1.9507x; 1.2642x over previous
"""Trainium2 Bass kernel for the CNF reversible backward solve.

Math restructuring (exact, validated in fp64 against the jax reference):

The per-step recursion is tracked purely in H-space (H=256) via
Z = W1 z, Y = W1 y:
    a_even = tanh(Y + beta_even)
    Z     += Mz @ a_even                       (Mz = -h W1 W2)
    a_odd  = tanh(Z + beta_odd)
    Y'     = inv_l Y + (1-inv_l) Z + inv_l Mz @ a_odd

On device both states live in PSUM banks and are updated by matmuls only:
  Z-bank carries Z + beta_odd(s) (biases injected by tiny rank-2 matmuls),
  so a_odd = tanh(Z-bank) directly.
  Y-bank carries Yhat_s = l*(Y_s + beta_even_s); with this normalization the
  update has STEP-CONSTANT weights:
      Yhat_{s+1} = Mz @ a_odd + (inv_l*I) @ p + rank2(delta_s)
      p = l(l-1) * Zbank + Yhat_s            (one fused DVE op, off-path)
  and a_even = tanh(inv_l * Y-bank).

The device streams all activations a_e to DRAM; the D-space outputs are
exact fp64 host-side postprocessing:
    y_final = c_y y1 + sum_e gamma_e (W2 @ a_e) + c_b b2
    I_final = h (N sum(c) - sum_s c . a_even_s^2),   c = diag(W1 W2)

Sharding: data-parallel, B=256 -> 32 samples on each of 8 cores; parameters
replicated; gather + assembly on host.
"""

import numpy as np
import ml_dtypes
from contextlib import ExitStack

import concourse.bass as bass
import concourse.tile as tile
from concourse import bacc, mybir
from concourse.bass_utils import run_bass_kernel_spmd

# Problem constants (hardcoded per contract)
NCORES = 8
B, D, H = 256, 64, 256
NSTEP = 64
HSTEP = 1.0 / NSTEP
LCOUP = 0.999
INVL = 1.0 / LCOUP
BS = B // NCORES  # 32 samples per core
NBLK = H // 128  # 2 h-blocks
FREE = NBLK * BS  # 64: free size of H-space tiles, layout (blk, sample)
NEVAL = 2 * NSTEP  # 128
ACOLS = NSTEP * FREE  # 4096 columns in each activation stream buffer
DMA_CHUNKS = 8
CSTEPS = NSTEP // DMA_CHUNKS  # steps per out-DMA chunk
CTIL = LCOUP * (LCOUP - 1.0)  # STT scalar on Z-bank

F32 = mybir.dt.float32
BF16 = mybir.dt.bfloat16
BF16NP = ml_dtypes.bfloat16

SHARED_INPUTS = [
    "w1t", "w1tl", "mzt", "mzl", "ib16", "dbz", "dby", "dbz0", "dby0", "ind", "indb",
]


def _coefficients():
    """Exact fp64 scalar recursions for the output-extraction weights."""
    gamma = np.zeros(NEVAL)
    la = np.zeros(NEVAL)
    alpha_y = alpha_z = 1.0
    nu_y = nu_z = 0.0
    for s in range(NSTEP):
        la[2 * s] += -HSTEP
        nu_z += -HSTEP
        gamma *= INVL
        alpha_y *= INVL
        nu_y *= INVL
        gamma += (1.0 - INVL) * la
        alpha_y += (1.0 - INVL) * alpha_z
        nu_y += (1.0 - INVL) * nu_z
        gamma[2 * s + 1] += -INVL * HSTEP
        nu_y += -INVL * HSTEP
    return gamma, alpha_y, nu_y


def _host_tables(W1, b1, u1, W2, b2):
    """All precomputed tensors, fp64 internally."""
    W1 = W1.astype(np.float64)
    W2 = W2.astype(np.float64)
    b1 = b1.astype(np.float64)
    u1 = u1.astype(np.float64)
    b2 = b2.astype(np.float64)

    Mz = -HSTEP * (W1 @ W2)  # [H, H]
    W1b2 = W1 @ b2  # [H]
    l = LCOUP

    def be(s):
        return b1 + (1.0 - s * HSTEP) * u1

    def bp(s):  # beta_odd
        return b1 + (1.0 - (s + 1) * HSTEP) * u1 - (s + 1) * HSTEP * W1b2

    # mzt_pack[p, (k*NBLK+j)*128 + m] = Mz[128*j+m, 128*k+p]
    MzT = Mz.T
    mzt_pack = np.zeros((128, NBLK * NBLK * 128))
    for k in range(NBLK):
        for j in range(NBLK):
            mzt_pack[:, (k * NBLK + j) * 128 : (k * NBLK + j + 1) * 128] = MzT[
                128 * k : 128 * k + 128, 128 * j : 128 * j + 128
            ]

    # rank-2 bias tables: lhsT slice [2, 128] at cols 128*s
    dbz = np.zeros((2, NSTEP * 128))
    dby = np.zeros((2, NSTEP * 128))
    for s in range(NSTEP):
        dz = bp(s) if s == 0 else bp(s) - bp(s - 1)
        for k in range(NBLK):
            dbz[k, s * 128 : (s + 1) * 128] = dz[128 * k : 128 * k + 128]
    for s in range(NSTEP - 1):
        dh = -HSTEP * W1b2 + l * be(s + 1) - (l - 1.0) * bp(s) - be(s)
        if s >= 1:
            # p' reads Z-bank BEFORE this step's delta; compensate here
            dh = dh + (l - 1.0) * (bp(s) - bp(s - 1))
        for k in range(NBLK):
            dby[k, s * 128 : (s + 1) * 128] = dh[128 * k : 128 * k + 128]
    # col-block NSTEP-1 of dby = init bias l*be(0)
    ib = l * be(0)
    for k in range(NBLK):
        dby[k, (NSTEP - 1) * 128 : NSTEP * 128] = ib[128 * k : 128 * k + 128]

    ind = np.zeros((2, FREE))
    for k in range(NBLK):
        ind[k, k * BS : (k + 1) * BS] = 1.0

    # init biases stay fp32 (full magnitude); per-step deltas are tiny -> bf16
    dbz0 = dbz[:, 0:128].astype(np.float32)
    dby0 = dby[:, (NSTEP - 1) * 128 : NSTEP * 128].astype(np.float32)

    return dict(
        mzt=mzt_pack.astype(BF16NP),
        mzl=((l - 1.0) * mzt_pack).astype(BF16NP),
        ib16=np.eye(128).astype(BF16NP),
        dbz=dbz.astype(BF16NP),
        dby=dby.astype(BF16NP),
        dbz0=dbz0,
        dby0=dby0,
        ind=ind.astype(np.float32),
        indb=ind.astype(BF16NP),
        w1t=W1.T.astype(np.float32),
        w1tl=(l * W1.T).astype(np.float32),
    )


def _build_kernel():
    """Build the Bass module (same program for every core)."""
    nc = bacc.Bacc("TRN2", target_bir_lowering=False, debug=False)

    y1t_d = nc.dram_tensor("y1t", [D, BS], F32, kind="ExternalInput").ap()
    w1t_d = nc.dram_tensor("w1t", [D, H], F32, kind="ExternalInput").ap()
    w1tl_d = nc.dram_tensor("w1tl", [D, H], F32, kind="ExternalInput").ap()
    mzt_d = nc.dram_tensor("mzt", [128, NBLK * NBLK * 128], BF16, kind="ExternalInput").ap()
    mzl_d = nc.dram_tensor("mzl", [128, NBLK * NBLK * 128], BF16, kind="ExternalInput").ap()
    ib16_d = nc.dram_tensor("ib16", [128, 128], BF16, kind="ExternalInput").ap()
    dbz_d = nc.dram_tensor("dbz", [2, NSTEP * 128], BF16, kind="ExternalInput").ap()
    dby_d = nc.dram_tensor("dby", [2, NSTEP * 128], BF16, kind="ExternalInput").ap()
    dbz0_d = nc.dram_tensor("dbz0", [2, 128], F32, kind="ExternalInput").ap()
    dby0_d = nc.dram_tensor("dby0", [2, 128], F32, kind="ExternalInput").ap()
    ind_d = nc.dram_tensor("ind", [2, FREE], F32, kind="ExternalInput").ap()
    indb_d = nc.dram_tensor("indb", [2, FREE], BF16, kind="ExternalInput").ap()

    ae_out_d = nc.dram_tensor("ae_out", [128, ACOLS], BF16, kind="ExternalOutput").ap()
    ao_out_d = nc.dram_tensor("ao_out", [128, ACOLS], BF16, kind="ExternalOutput").ap()

    with tile.TileContext(nc) as tc, ExitStack() as ctx:
        consts = ctx.enter_context(tc.tile_pool(name="consts", bufs=1))
        zpool = ctx.enter_context(tc.tile_pool(name="zps", bufs=1, space="PSUM"))
        ypool = ctx.enter_context(tc.tile_pool(name="yps", bufs=2, space="PSUM"))
        ppool = ctx.enter_context(tc.tile_pool(name="ptmp", bufs=2))

        # --- prime the tanh activation table early (dep-free) ---
        warm = consts.tile([1, 8], F32, tag="warm")
        nc.vector.memset(warm[:], 0.0)
        nc.scalar.activation(warm[:], warm[:], mybir.ActivationFunctionType.Tanh)

        # --- load constants ---
        y1t = consts.tile([D, BS], F32, tag="y1t")
        nc.sync.dma_start(y1t[:], y1t_d)
        w1t = consts.tile([D, H], F32, tag="w1t")
        nc.sync.dma_start(w1t[:], w1t_d)
        w1tl = consts.tile([D, H], F32, tag="w1tl")
        nc.sync.dma_start(w1tl[:], w1tl_d)
        mzt = consts.tile([128, NBLK * NBLK * 128], BF16, tag="mzt")
        nc.sync.dma_start(mzt[:], mzt_d)
        mzl = consts.tile([128, NBLK * NBLK * 128], BF16, tag="mzl")
        nc.sync.dma_start(mzl[:], mzl_d)
        ib16 = consts.tile([128, 128], BF16, tag="ib16")
        nc.sync.dma_start(ib16[:], ib16_d)
        dbz = consts.tile([2, NSTEP * 128], BF16, tag="dbz")
        nc.sync.dma_start(dbz[:], dbz_d)
        dby = consts.tile([2, NSTEP * 128], BF16, tag="dby")
        nc.sync.dma_start(dby[:], dby_d)
        dbz0 = consts.tile([2, 128], F32, tag="dbz0")
        nc.sync.dma_start(dbz0[:], dbz0_d)
        dby0 = consts.tile([2, 128], F32, tag="dby0")
        nc.sync.dma_start(dby0[:], dby0_d)
        ind = consts.tile([2, FREE], F32, tag="ind")
        nc.sync.dma_start(ind[:], ind_d)
        indb = consts.tile([2, FREE], BF16, tag="indb")
        nc.sync.dma_start(indb[:], indb_d)

        # --- activation stream buffers (one tile per DMA chunk so the
        # out-DMA of chunk c never WAR-blocks ACT writes of chunk c+1) ---
        CCOLS = CSTEPS * FREE
        abuf_e = [
            consts.tile([128, CCOLS], BF16, tag=f"abe{c}", name=f"abe{c}")
            for c in range(DMA_CHUNKS)
        ]
        abuf_o = [
            consts.tile([128, CCOLS], BF16, tag=f"abo{c}", name=f"abo{c}")
            for c in range(DMA_CHUNKS)
        ]

        def mzt_blk(k, j):
            base = (k * NBLK + j) * 128
            return mzt[:, base : base + 128]

        def mzl_blk(k, j):
            base = (k * NBLK + j) * 128
            return mzl[:, base : base + 128]

        z_ps = zpool.tile([128, FREE], F32, tag="z")

        # --- init Z-bank = W1 @ y1 + beta_odd(0) ---
        for j in range(NBLK):
            nc.tensor.matmul(
                z_ps[:, j * BS : (j + 1) * BS],
                w1t[:, 128 * j : 128 * j + 128],
                y1t[:],
                start=(j == 0),
                stop=False,
            )
        nc.tensor.matmul(z_ps[:], dbz0[:], ind[:], start=False, stop=True)

        # --- init Y-bank = l*(W1 @ y1) + l*be(0) ---
        y_cur = ypool.tile([128, FREE], F32, tag="y")
        for j in range(NBLK):
            nc.tensor.matmul(
                y_cur[:, j * BS : (j + 1) * BS],
                w1tl[:, 128 * j : 128 * j + 128],
                y1t[:],
                start=(j == 0),
                stop=False,
            )
        nc.tensor.matmul(y_cur[:], dby0[:], ind[:], start=False, stop=True)

        for s in range(NSTEP):
            last = s == NSTEP - 1
            chunk, cstep = divmod(s, CSTEPS)
            ecol = cstep * FREE

            if not last:
                # t1 = (l-1) * Zbank_pre, read BEFORE this step's delta-MM
                # (hidden: runs during the previous step's tail)
                t_t = ppool.tile([128, FREE], F32, tag="t")
                nc.vector.tensor_scalar_mul(t_t[:], z_ps[:], LCOUP - 1.0)

            # Z-bank bias delta for THIS step's odd read
            if s > 0:
                nc.tensor.matmul(
                    z_ps[:], dbz[:, s * 128 : (s + 1) * 128], indb[:],
                    start=False, stop=False, skip_group_check=True,
                )

            # --- even eval: a_even = tanh(inv_l * Ybank) (single wide ACT) ---
            a_even = abuf_e[chunk][:, ecol : ecol + FREE]
            nc.scalar.activation(
                a_even[:], y_cur[:], mybir.ActivationFunctionType.Tanh, scale=INVL
            )

            if not last:
                # --- p = inv_l Ybank + t1; compensated bf16 split p = hi+lo
                # (all DVE, hidden under ACTs / matmul groups) ---
                p_t = ppool.tile([128, FREE], F32, tag="p")
                nc.vector.scalar_tensor_tensor(
                    p_t[:], y_cur[:], INVL, t_t[:],
                    mybir.AluOpType.mult, mybir.AluOpType.add,
                )
                p_hi = ppool.tile([128, FREE], BF16, tag="phi")
                nc.vector.tensor_copy(p_hi[:], p_t[:])
                p_lo = ppool.tile([128, FREE], BF16, tag="plo")
                nc.vector.scalar_tensor_tensor(
                    p_lo[:], p_hi[:], -1.0, p_t[:],
                    mybir.AluOpType.mult, mybir.AluOpType.add,
                )

            # --- Z += Mz @ a_even ---
            for j in range(NBLK):
                for k in range(NBLK):
                    nc.tensor.matmul(
                        z_ps[:, j * BS : (j + 1) * BS],
                        mzt_blk(k, j),
                        a_even[:, k * BS : (k + 1) * BS],
                        start=False,
                        stop=False,
                        skip_group_check=True,
                    )

            if not last:
                # open next Y-bank with its bias (hidden, dep-free), then the
                # a_even-driven part (l-1) Mz @ a_even (also hidden: only
                # needs a_even, runs during the odd ACT)
                y_next = ypool.tile([128, FREE], F32, tag="y")
                nc.tensor.matmul(
                    y_next[:], dby[:, s * 128 : (s + 1) * 128], indb[:],
                    start=True, stop=False,
                )
                for j in range(NBLK):
                    for k in range(NBLK):
                        nc.tensor.matmul(
                            y_next[:, j * BS : (j + 1) * BS],
                            mzl_blk(k, j),
                            a_even[:, k * BS : (k + 1) * BS],
                            start=False,
                            stop=False,
                        )

            # --- odd eval: a_odd = tanh(Zbank) (single wide ACT) ---
            a_odd = abuf_o[chunk][:, ecol : ecol + FREE]
            nc.scalar.activation(
                a_odd[:], z_ps[:], mybir.ActivationFunctionType.Tanh, scale=1.0
            )

            if not last:
                # --- Ynext += Mz @ a_odd + I @ p_hi + I @ p_lo ---
                for j in range(NBLK):
                    for k in range(NBLK):
                        nc.tensor.matmul(
                            y_next[:, j * BS : (j + 1) * BS],
                            mzt_blk(k, j),
                            a_odd[:, k * BS : (k + 1) * BS],
                            start=False,
                            stop=False,
                        )
                nc.tensor.matmul(y_next[:], ib16[:], p_hi[:], start=False, stop=False)
                nc.tensor.matmul(y_next[:], ib16[:], p_lo[:], start=False, stop=True)
                y_cur = y_next

            # --- stream out completed chunks ---
            if (s + 1) % CSTEPS == 0:
                c0 = chunk * CCOLS
                nc.sync.dma_start(ae_out_d[:, c0 : c0 + CCOLS], abuf_e[chunk][:])
                nc.sync.dma_start(ao_out_d[:, c0 : c0 + CCOLS], abuf_o[chunk][:])

    nc.compile()
    return nc


_CACHE = {}


def _get_kernel():
    if "nc" not in _CACHE:
        _CACHE["nc"] = _build_kernel()
    return _CACHE["nc"]


def kernel(y1, W1, b1, u1, W2, b2, _trace=False, _trace_kwargs=None):
    y1 = np.asarray(y1)
    in_dtype = y1.dtype
    W1_ = np.asarray(W1, dtype=np.float64)
    W2_ = np.asarray(W2, dtype=np.float64)
    b2_ = np.asarray(b2, dtype=np.float64)
    tabs = _host_tables(
        np.asarray(W1), np.asarray(b1), np.asarray(u1), np.asarray(W2), np.asarray(b2)
    )

    nc = _get_kernel()

    shared = {k: tabs[k] for k in SHARED_INPUTS}
    in_maps = []
    for c in range(NCORES):
        shard = y1[c * BS : (c + 1) * BS].astype(np.float32)  # [BS, D]
        m = dict(shared)
        m["y1t"] = np.ascontiguousarray(shard.T)  # [D, BS]
        in_maps.append(m)

    kw = {}
    if _trace:
        kw["trace"] = True
        if _trace_kwargs:
            kw.update(_trace_kwargs)
    res = run_bass_kernel_spmd(nc, in_maps, core_ids=list(range(NCORES)), **kw)

    # --- exact host-side output extraction ---
    gamma, c_y, c_b = _coefficients()
    cvec = np.sum(W1_ * W2_.T, axis=1)  # diag(W1@W2)
    sum_c = float(np.sum(cvec))

    out = np.zeros((B, D + 1), dtype=np.float32)
    for c in range(NCORES):
        ae = np.asarray(res.results[c]["ae_out"]).astype(np.float64)  # [128, ACOLS]
        ao = np.asarray(res.results[c]["ao_out"]).astype(np.float64)
        ae = ae.reshape(128, NSTEP, NBLK, BS)  # [p, s, blk, b]
        ao = ao.reshape(128, NSTEP, NBLK, BS)
        ae = np.moveaxis(ae, (2, 0), (1, 2)).reshape(NSTEP, H, BS)  # [s, h, b]
        ao = np.moveaxis(ao, (2, 0), (1, 2)).reshape(NSTEP, H, BS)

        S = np.einsum("s,shb->hb", gamma[0::2], ae) + np.einsum(
            "s,shb->hb", gamma[1::2], ao
        )
        shard = y1[c * BS : (c + 1) * BS].astype(np.float64)  # [BS, D]
        y_fin = c_y * shard + (W2_ @ S).T + c_b * b2_[None, :]
        ptr = np.einsum("h,shb->b", cvec, ae**2)
        i_fin = HSTEP * (NSTEP * sum_c - ptr)
        out[c * BS : (c + 1) * BS, :D] = y_fin.astype(np.float32)
        out[c * BS : (c + 1) * BS, D] = i_fin.astype(np.float32)

    if _trace:
        return out.astype(in_dtype, copy=False), res
    return out.astype(in_dtype, copy=False)
